# revision 1
# baseline (speedup 1.0000x reference)
"""RGCN (2-layer, basis-decomposition) Trainium2 kernel, v2.

Strategy (8 NeuronCores, SPMD), building on v1:
  - Edges sorted by destination; dst nodes partitioned into 8 contiguous
    ranges (one per core). Messages + segment-sum fused into per-tile PE
    matmuls against a DVE-built "weighted one-hot" (woh) matrix.
  - NEW in v2: layer 1's per-edge source rows are pre-gathered ON THE HOST
    into tile-slot order (the gather indices are static), so layer 1 does
    dense sequential DMA loads instead of gpsimd dma_gather (which costs
    ~8 ns/row of Q7 descriptor generation). This removes the parity trick
    for layer 1: one matmul per tile, 4 instead of 8 woh multiplies.
  - NEW in v2: the inter-layer AllGather of h is split into 4 chunks, each
    fired as soon as its groups' h rows are written -- overlapping the
    collective with layer-1 tail compute (gpsimd is idle during layer 1).
  - Layer 2 keeps the dma_gather path (h is device-computed): bf16 rows
    paired into 512-B table rows to fit int16 gather indices; even/odd
    parity handled by two matmuls per tile against parity-masked wohs.
"""

import math

import numpy as np
import ml_dtypes

import concourse.bacc as bacc
import concourse.bass as bass
import concourse.mybir as mybir
import concourse.tile as tile
from concourse.bass_utils import run_bass_kernel_spmd

F32 = mybir.dt.float32
BF16 = mybir.dt.bfloat16
I16 = mybir.dt.int16
AF = mybir.ActivationFunctionType
ALU = mybir.AluOpType
BF = ml_dtypes.bfloat16

M = 8            # cores
BLK = 32         # nodes per scatter block
GRP = 128        # nodes per output group (4 blocks)
TPE = 128        # edges per tile
G = 32           # tiles per gather page
CHK = 8          # tiles per wonehot build chunk
DW = 128         # padded table row width (256 B in bf16)
BLK2 = 64        # layer-2 block size (fewer tiles -> fewer gather descriptors)
CAPW = 4         # base tiles per BLK2 block (overflow goes to spill tiles)
NCHUNK = 4       # AllGather chunks


def _expand(ap, free_dims, col_offset=0):
    """AP with the partition dim kept and explicit [step, count] free dims."""
    base = ap.ap
    return bass.AP(
        ap.tensor,
        ap.offset + col_offset,
        [list(base[0])] + [list(d) for d in free_dims],
    )


def _prep(N, D, entity, edge_index, edge_type, edge_norm, att1, att2):
    """Host-side graph preprocessing. Returns per-core arrays + structure."""
    NPC = int(math.ceil(N / (M * GRP))) * GRP      # nodes per core (6272)
    SEG = NPC + GRP                                # table segment per core (6400)
    NTAB = SEG * M                                 # 51200
    NBLK = NPC // BLK
    NGRP = NPC // GRP

    # AllGather chunking of the 49 groups: sizes ~equal; last chunk also
    # holds the zero-row region (GRP extra nodes).
    base_cg = NGRP // NCHUNK
    cg = [base_cg] * NCHUNK
    cg[0] += NGRP - base_cg * NCHUNK               # [13,12,12,12]
    chunk_of_group = np.repeat(np.arange(NCHUNK), cg)
    chunk_gstart = np.concatenate([[0], np.cumsum(cg)[:-1]])
    pr_c = [x * (GRP // 2) for x in cg]
    pr_c[0] += GRP // 2                            # zero region pair rows
    hfull_base = np.concatenate([[0], np.cumsum([p * M for p in pr_c])[:-1]])
    zero_row = int(hfull_base[0] + pr_c[0] - GRP // 2)  # core 0's zero region

    src = np.asarray(edge_index[0], dtype=np.int64)
    dst = np.asarray(edge_index[1], dtype=np.int64)
    et = np.asarray(edge_type[:, 0], dtype=np.int64)
    norm = np.asarray(edge_norm, dtype=np.float32)

    order = np.argsort(dst, kind="stable")
    src_s, dst_s, et_s, norm_s = src[order], dst[order], et[order], norm[order]

    deg = np.bincount(dst, minlength=NPC * M).astype(np.float32)
    inv_deg = (1.0 / np.maximum(deg, 1.0)).astype(np.float32)

    def tilestruct(BLKx):
        """Per-layer tile structure for block size BLKx."""
        nblk = NPC // BLKx
        gb = dst_s // BLKx
        cnt = np.bincount(gb, minlength=nblk * M).reshape(M, nblk)
        T_k = np.maximum(1, -(-cnt // TPE)).max(axis=0)
        T_total = int(T_k.sum())
        npage = -(-T_total // G)
        T_k[-1] += npage * G - T_total
        T = npage * G
        tb = np.zeros(nblk, dtype=np.int64)
        tb[1:] = np.cumsum(T_k)[:-1]
        lb = gb % nblk
        es = np.zeros(nblk * M + 1, dtype=np.int64)
        es[1:] = np.cumsum(np.bincount(gb, minlength=nblk * M))
        within = np.arange(len(dst_s)) - es[gb]
        slot = tb[lb] * TPE + within
        tiles = []
        for k in range(nblk):
            for t in range(int(T_k[k])):
                tiles.append((k, t == 0, t == int(T_k[k]) - 1))
        return dict(slot=slot, T=T, NPAGE=npage, tiles=tiles,
                    doff=(dst_s % BLKx).astype(np.float32))

    st1 = tilestruct(BLK)
    core_of = dst_s // NPC

    # ---- layer-2 spill structure: per block-pair (BLK2) 4 base tiles,
    # plus one group-wide spill tile per group; spans of 2 groups ----
    CAP = CAPW                    # base tiles per BLK2 block
    G2 = 34                       # tiles per layer-2 page (2 spans of 17)
    nblk2 = NPC // BLK2
    # chunk/tile schedule (shared by host packing and kernel build)
    l2_tiles = []                 # per tile: (kind, g, bi, w)
    l2_chunks = []                # per chunk: (kind, CHKs, tile_off_global)
    for j in range((NGRP + 1) // 2):
        gs = [2 * j] + ([2 * j + 1] if 2 * j + 1 < NGRP else [])
        l2_chunks.append(("s", 1, len(l2_tiles)))
        l2_tiles.append(("s", j, len(gs), 0))
        for half in range(len(gs)):
            g = gs[half]
            l2_chunks.append(("b", 8, len(l2_tiles)))
            for bi in range(2):
                for w in range(CAP):
                    l2_tiles.append(("b", g, bi, w))
    T2 = len(l2_tiles)            # 441
    NPAGE2 = -(-T2 // G2)         # 13
    l2_page_tiles = [min(G2, T2 - p * G2) for p in range(NPAGE2)]
    # base tile slot ranges: tile index of (block k, w)
    base_tile_id = np.zeros((nblk2, CAP), dtype=np.int64)
    spill_tile_id = np.zeros(NGRP, dtype=np.int64)
    for t, (kind, g, bi, w) in enumerate(l2_tiles):
        if kind == "s":
            spill_tile_id[2 * g] = t
            if 2 * g + 1 < NGRP:
                spill_tile_id[2 * g + 1] = t
        else:
            base_tile_id[2 * g + bi, w] = t

    nd = norm_s * inv_deg[dst_s]
    c1 = (att1[et_s] * nd[:, None]).astype(np.float32)
    c2 = (att2[et_s] * nd[:, None]).astype(np.float32)

    # layer-2 gather: chunk-major hfull pair-row index for each source node
    k_s = src_s // NPC
    n_s = src_s % NPC
    g_s = n_s // GRP
    c_s = chunk_of_group[g_s]
    lpr = (g_s - chunk_gstart[c_s]) * (GRP // 2) + (n_s % GRP) // 2
    row_s = hfull_base[c_s] + k_s * np.asarray(pr_c)[c_s] + lpr
    par_s = (n_s % 2).astype(np.float32)

    def pack_idx(lin, npage):
        """[npage*G2*TPE] indices -> [npage, 128, G2*8] int16 gather layout."""
        out = np.empty((npage, 16, G2 * 8), np.int16)
        lp = lin.reshape(npage, G2 * TPE)
        out[:, :, :] = lp.reshape(npage, G2 * 8, 16).transpose(0, 2, 1)
        return np.ascontiguousarray(np.tile(out, (1, 8, 1)))

    def scat(vals, sl, T, fill=0.0, width=None):
        if width is None:
            out = np.full(T * TPE, fill, dtype=np.float32)
        else:
            out = np.full((T * TPE, width), fill, dtype=np.float32)
        out[sl] = vals
        return out

    def tilemaj(a, npage, w, Gx=G):
        """[T*TPE(,w)] slot-major -> [npage, TPE, Gx*w] page/tile-major bf16."""
        if a.ndim == 1:
            a = a[:, None]
        return np.ascontiguousarray(
            a.reshape(npage, Gx, TPE, a.shape[1]).transpose(0, 2, 1, 3)
            .reshape(npage, TPE, Gx * a.shape[1]).astype(BF))

    ent = np.asarray(entity, dtype=np.float32)

    metac1s, metac2s, idx_pages, xpgs = [], [], [], []
    for m in range(M):
        sel = core_of == m
        sl1 = st1["slot"][sel]
        T1, NP1 = st1["T"], st1["NPAGE"]

        # layer-1 pre-gathered source rows, page layout [NP1, 128, G*D]
        xsrcm = np.zeros((T1 * TPE, D), dtype=BF)
        xsrcm[sl1] = ent[src_s[sel]].astype(BF)
        xpgs.append(tilemaj(xsrcm.astype(np.float32), NP1, D))

        # layer 1 meta: no parity; cols [0:G] doff, [G:5G] coeffs
        metac1 = np.concatenate([
            tilemaj(scat(st1["doff"][sel], sl1, T1, 99.0), NP1, 1),
            tilemaj(scat(c1[sel], sl1, T1, width=4), NP1, 4)], axis=2)
        metac1s.append(np.ascontiguousarray(metac1))

        # layer-2 slot assignment: base tiles per block (cap CAP*TPE edges),
        # overflow into the group's spill tile
        eidx = np.nonzero(sel)[0]
        gb2 = dst_s[eidx] // BLK2
        es2 = np.zeros(nblk2 + 1, dtype=np.int64)
        es2[1:] = np.cumsum(np.bincount(gb2 - m * nblk2, minlength=nblk2))
        w2 = np.arange(len(eidx)) - es2[gb2 - m * nblk2]
        lb2 = gb2 - m * nblk2
        is_base = w2 < CAP * TPE
        sl2 = np.empty(len(eidx), dtype=np.int64)
        sl2[is_base] = (base_tile_id[lb2[is_base], w2[is_base] // TPE] * TPE
                        + w2[is_base] % TPE)
        spill_e = ~is_base
        sg = lb2[spill_e] // 2
        # spill slots shared per span (4 blocks): offset each block's spill
        # edges by the spill counts of preceding blocks in its span
        sp_cnt = np.maximum(0, np.diff(es2) - CAP * TPE)
        tot = np.cumsum(sp_cnt) - sp_cnt
        span_of_blk = np.arange(nblk2) // 4
        first = np.searchsorted(span_of_blk, np.arange(span_of_blk[-1] + 1))
        sp_base = tot - tot[first][span_of_blk]
        sw = w2[spill_e] - CAP * TPE + sp_base[lb2[spill_e]]
        assert sw.max(initial=0) < TPE, sw.max(initial=0)
        sl2[spill_e] = spill_tile_id[sg] * TPE + sw
        TSLOTS = NPAGE2 * G2
        rows = np.full(TSLOTS * TPE, zero_row, dtype=np.int64)
        rows[sl2] = row_s[eidx]
        idx_pages.append(pack_idx(rows.astype(np.int16), NPAGE2))
        par = scat(par_s[eidx], sl2, TSLOTS)

        do2 = np.where(is_base, dst_s[eidx] % BLK2,
                       (dst_s[eidx] % NPC) % (2 * GRP))
        ca = scat(c2[eidx], sl2, TSLOTS, width=4)
        metac2 = np.concatenate([
            tilemaj(scat(do2.astype(np.float32), sl2, TSLOTS, 320.0), NPAGE2, 1, G2),
            tilemaj(ca * (1.0 - par)[:, None], NPAGE2, 4, G2),
            tilemaj(ca * par[:, None], NPAGE2, 4, G2)], axis=2)
        metac2s.append(np.ascontiguousarray(metac2))


    return dict(NPC=NPC, SEG=SEG, NTAB=NTAB, NGRP=NGRP,
                NPAGE1=st1["NPAGE"], NPAGE2=NPAGE2, G2=G2,
                tiles1=st1["tiles"], l2_tiles=l2_tiles, l2_chunks=l2_chunks,
                l2_page_tiles=l2_page_tiles,
                cg=cg, chunk_of_group=chunk_of_group,
                chunk_gstart=chunk_gstart, pr_c=pr_c, hfull_base=hfull_base,
                idxp=idx_pages, metac1=metac1s, metac2=metac2s, xpg=xpgs)


TAIL_DELAY = 5   # chunks between a group's last matmul and its tail emission
TAIL_DELAY2 = 2  # layer-2 variant (faster wind-down at the end)


def _layer1(tc, nc, pools, prm, D, xpg, metap, iota_t, ident_t,
            bas_t, rt_t, bias_t, xslb, h4buf, hsl_cs, group_done):
    """Dense layer 1: pre-gathered x rows, single matmul per tile."""
    meta, gath, wohp, hp, xtp, sbigp, psp = pools
    NPAGE, tile_blocks = prm["NPAGE1"], prm["tiles1"]

    pending = []

    def flush(now):
        while pending and (now is None or pending[0][0] + TAIL_DELAY <= now):
            _, g, sbig = pending.pop(0)
            _tail1(tc, nc, pools, prm, D, g, sbig, xslb, h4buf,
                   ident_t, bas_t, rt_t, bias_t, hsl_cs)
            group_done(g)

    tcount = 0
    psum_blk = None
    for q in range(NPAGE):
        metat = meta.tile([TPE, 5 * G], BF16, tag="metat1")
        nc.sync.dma_start(out=metat[:], in_=metap[q])
        gbuf = gath.tile([TPE, G * D], BF16, tag="gbuf1", bufs=4)
        nc.sync.dma_start(out=gbuf[:], in_=xpg[q])
        for c in range(G // CHK):
            cglob = q * (G // CHK) + c
            flush(cglob)
            oh8 = wohp.tile([TPE, CHK * BLK], BF16, tag="oh1", bufs=5)
            nc.vector.tensor_tensor(
                out=_expand(oh8[:], [[BLK, CHK], [1, BLK]]),
                in0=iota_t,
                in1=_expand(metat[:], [[1, CHK], [0, BLK]],
                            col_offset=c * CHK),
                op=ALU.is_equal,
            )
            woh = wohp.tile([TPE, CHK * 4 * BLK], BF16, tag="woh1", bufs=5)
            for b in range(4):
                nc.vector.tensor_tensor(
                    out=_expand(woh[:], [[4 * BLK, CHK], [1, BLK]],
                                col_offset=b * BLK),
                    in0=_expand(oh8[:], [[BLK, CHK], [1, BLK]]),
                    in1=_expand(metat[:], [[4, CHK], [0, BLK]],
                                col_offset=G + c * CHK * 4 + b),
                    op=ALU.mult,
                )
            for u in range(CHK):
                blk, first, last = tile_blocks[tcount]
                g, bi = blk // 4, blk % 4
                if first and bi == 0:
                    psum_blk = psp.tile([D, 4 * GRP], F32, tag="blk", bufs=3)
                out_sl = psum_blk[:, bi * 4 * BLK:(bi + 1) * 4 * BLK]
                nc.tensor.matmul(out=out_sl,
                                 lhsT=gbuf[:, (c * CHK + u) * D:(c * CHK + u + 1) * D],
                                 rhs=woh[:, u * 4 * BLK:(u + 1) * 4 * BLK],
                                 start=first, stop=last)
                if last and bi == 3:
                    sbig = sbigp.tile([D, 4 * GRP], BF16, tag="sbig", bufs=6)
                    for b in range(4):
                        nc.scalar.copy(
                            out=_expand(sbig[:], [[BLK, 4], [1, BLK]],
                                        col_offset=b * GRP),
                            in_=_expand(psum_blk[:], [[4 * BLK, 4], [1, BLK]],
                                        col_offset=b * BLK),
                        )
                    pending.append((cglob, g, sbig))
                tcount += 1
    flush(None)


def _tail1(tc, nc, pools, prm, D, g, sbig, xslb, h4buf, ident_t,
           bas_t, rt_t, bias_t, hsl_cs):
    """Layer-1 group tail: combine bases, mean, root, bias, relu -> hsl chunk."""
    meta, gath, wohp, hp, xtp, sbigp, psp = pools
    pagg = psp.tile([GRP, D], F32, tag="agg", bufs=2)
    for b in range(4):
        nc.tensor.matmul(
            out=pagg[:],
            lhsT=sbig[:, b * GRP:(b + 1) * GRP],
            rhs=bas_t[:, b * D:(b + 1) * D],
            start=(b == 0),
            stop=False,
        )
    nc.tensor.matmul(out=pagg[:], lhsT=xslb[:, g, :], rhs=rt_t,
                     start=False, stop=True)

    h3 = hp.tile([GRP, D], F32, tag="h3")
    nc.vector.tensor_tensor(out=h3[:], in0=pagg[:], in1=bias_t, op=ALU.add)

    nc.scalar.activation(out=h4buf[:, g, 0:D], in_=h3[:], func=AF.Relu)
    c = int(prm["chunk_of_group"][g])
    gl = g - int(prm["chunk_gstart"][c])
    base = hsl_cs[c][:]
    dst = bass.AP(base.tensor, base.offset + gl * (GRP // 2) * (2 * DW),
                  [[2 * DW, GRP // 2], [DW, 2], [1, DW]])
    nc.sync.dma_start(out=dst, in_=h4buf[:, g, :])


def _layer2(tc, nc, pools, prm, D, table_ap, idxp, metap,
            iota_t, iota_s, ident_t, bas_t, rt_t, bias_t, h4buf,
            out_ap):
    """Gather-based layer 2: per-block base tiles + per-group spill tiles."""
    meta, gath, wohp, hp, xtp, sbigp, psp = pools
    NPAGE, G2 = prm["NPAGE2"], prm["G2"]
    l2_tiles, l2_chunks = prm["l2_tiles"], prm["l2_chunks"]
    page_tiles = prm["l2_page_tiles"]

    pending = []

    def flush(now):
        while pending and (now is None or pending[0][0] + TAIL_DELAY2 <= now):
            _, g, sbig = pending.pop(0)
            _tail2(tc, nc, pools, prm, D, g, sbig, h4buf, ident_t,
                   bas_t, rt_t, bias_t, out_ap)

    # split chunks by page (a chunk never crosses a page boundary:
    # pages are 36 tiles = 2 spans of 18, chunks are 2+8+8 per span)
    chunks_by_page = [[] for _ in range(NPAGE)]
    for kind, chks, toff in l2_chunks:
        chunks_by_page[toff // G2].append((kind, chks, toff))

    psums = {}
    cglob = 0
    for q in range(NPAGE):
        nt = page_tiles[q]
        nidx = nt * TPE
        idxt = meta.tile([TPE, 8 * G2], I16, tag="idxt", bufs=4)
        nc.sync.dma_start(out=idxt[:], in_=idxp[q])
        metat = meta.tile([TPE, 9 * G2], BF16, tag="metat2", bufs=4)
        nc.scalar.dma_start(out=metat[:], in_=metap[q])
        gbuf = gath.tile([TPE, G2, 2 * DW], BF16, tag="gbuf2", bufs=4)
        nc.gpsimd.dma_gather(
            out_ap=gbuf[:, 0:nt, :], in_ap=table_ap,
            idxs_ap=idxt[:, 0:nidx // 16], num_idxs=nidx, num_idxs_reg=nidx,
            elem_size=2 * DW, single_packet=False)
        for kind, chks, toff in chunks_by_page[q]:
            flush(cglob)
            cglob += 1
            loff = toff - q * G2
            BLKx = 2 * GRP if kind == "s" else BLK2
            iot = iota_s if kind == "s" else iota_t
            oh8 = wohp.tile([TPE, 8 * BLK2], BF16, tag="oh2", bufs=7)
            nc.vector.tensor_tensor(
                out=_expand(oh8[:], [[BLKx, chks], [1, BLKx]]),
                in0=iot[:, 0:chks * BLKx],
                in1=_expand(metat[:], [[1, chks], [0, BLKx]],
                            col_offset=loff),
                op=ALU.is_equal,
            )
            wohE = wohp.tile([TPE, 8 * 4 * BLK2], BF16, tag="wohE", bufs=6)
            wohO = wohp.tile([TPE, 8 * 4 * BLK2], BF16, tag="wohO", bufs=6)
            for woh, cbase in ((wohE, G2), (wohO, 5 * G2)):
                for b in range(4):
                    if kind == "s":
                        # spill (span): col ghalf*512 + bi*256 + b*64 + j%64,
                        # matching the base tiles' psum layout
                        nc.vector.tensor_tensor(
                            out=_expand(woh[:],
                                        [[8 * BLK2, 2], [4 * BLK2, 2],
                                         [1, BLK2]],
                                        col_offset=b * BLK2),
                            in0=_expand(oh8[:],
                                        [[2 * BLK2, 2], [BLK2, 2], [1, BLK2]]),
                            in1=_expand(metat[:],
                                        [[0, 2], [0, 2], [0, BLK2]],
                                        col_offset=cbase + loff * 4 + b),
                            op=ALU.mult,
                        )
                    else:
                        nc.vector.tensor_tensor(
                            out=_expand(woh[:], [[4 * BLKx, chks], [1, BLKx]],
                                        col_offset=b * BLKx),
                            in0=_expand(oh8[:], [[BLKx, chks], [1, BLKx]]),
                            in1=_expand(metat[:], [[4, chks], [0, BLKx]],
                                        col_offset=cbase + loff * 4 + b),
                            op=ALU.mult,
                        )
            for u in range(chks):
                kindt, g, bi, w = l2_tiles[toff + u]
                lt = loff + u
                if kindt == "s":
                    for gh in range(bi):       # bi = ngroups in this span
                        gg = 2 * g + gh
                        psums[gg] = psp.tile([D, 4 * GRP], F32, tag="blk",
                                             bufs=3, name="psum_blk")
                        nc.tensor.matmul(
                            out=psums[gg][:, :], lhsT=gbuf[:, lt, 0:D],
                            rhs=wohE[:, gh * 512:(gh + 1) * 512],
                            start=True, stop=False)
                        nc.tensor.matmul(
                            out=psums[gg][:, :], lhsT=gbuf[:, lt, DW:DW + D],
                            rhs=wohO[:, gh * 512:(gh + 1) * 512],
                            start=False, stop=False)
                    continue
                out_sl = psums[g][:, bi * 4 * BLK2:(bi + 1) * 4 * BLK2]
                first, stop = False, (w == CAPW - 1)
                nc.tensor.matmul(out=out_sl, lhsT=gbuf[:, lt, 0:D],
                                 rhs=wohE[:, u * 4 * BLKx:(u + 1) * 4 * BLKx],
                                 start=first, stop=False)
                nc.tensor.matmul(out=out_sl,
                                 lhsT=gbuf[:, lt, DW:DW + D],
                                 rhs=wohO[:, u * 4 * BLKx:(u + 1) * 4 * BLKx],
                                 start=False, stop=stop)
                if kindt == "b" and bi == 1 and w == CAPW - 1:
                    psum_blk = psums.pop(g)
                    sbig = sbigp.tile([D, 4 * GRP], BF16, tag="sbig", bufs=6)
                    for b in range(4):
                        nc.scalar.copy(
                            out=_expand(sbig[:], [[BLK2, 2], [1, BLK2]],
                                        col_offset=b * GRP),
                            in_=_expand(psum_blk[:], [[4 * BLK2, 2], [1, BLK2]],
                                        col_offset=b * BLK2),
                        )
                    pending.append((cglob, g, sbig))
    flush(None)


def _tail2(tc, nc, pools, prm, D, g, sbig, h4buf, ident_t,
           bas_t, rt_t, bias_t, out_ap):
    meta, gath, wohp, hp, xtp, sbigp, psp = pools
    ptr = psp.tile([D, GRP], BF16, tag="tr", bufs=2)
    nc.tensor.transpose(out=ptr[:], in_=h4buf[:, g, 0:D], identity=ident_t)
    xt = xtp.tile([D, GRP], BF16, tag="xt")
    nc.scalar.copy(out=xt[:], in_=ptr[:])

    pagg = psp.tile([GRP, D], F32, tag="agg", bufs=2)
    for b in range(4):
        nc.tensor.matmul(
            out=pagg[:],
            lhsT=sbig[:, b * GRP:(b + 1) * GRP],
            rhs=bas_t[:, b * D:(b + 1) * D],
            start=(b == 0),
            stop=False,
        )
    nc.tensor.matmul(out=pagg[:], lhsT=xt[:], rhs=rt_t, start=False, stop=True)

    h3 = hp.tile([GRP, D], F32, tag="h3")
    nc.vector.tensor_tensor(out=h3[:], in0=pagg[:], in1=bias_t, op=ALU.add)
    nc.scalar.dma_start(out=out_ap[g * GRP:(g + 1) * GRP, :], in_=h3[:])


def _build(prm, D):
    NPC, NTAB = prm["NPC"], prm["NTAB"]
    NPAGE1, NPAGE2, NGRP = prm["NPAGE1"], prm["NPAGE2"], prm["NGRP"]
    pr_c, cg = prm["pr_c"], prm["cg"]
    nc = bacc.Bacc()

    xslp = nc.dram_tensor("xslp", [D, NGRP * GRP], BF16, kind="ExternalInput")
    xpg = nc.dram_tensor("xpg", [NPAGE1, TPE, G * D], BF16, kind="ExternalInput")
    G2 = prm["G2"]
    idxp = nc.dram_tensor("idxp", [NPAGE2, TPE, 8 * G2], I16, kind="ExternalInput")
    metac1 = nc.dram_tensor("metac1", [NPAGE1, TPE, 5 * G], BF16, kind="ExternalInput")
    metac2 = nc.dram_tensor("metac2", [NPAGE2, TPE, 9 * G2], BF16, kind="ExternalInput")
    iota = nc.dram_tensor("iota", [TPE, CHK * BLK], BF16, kind="ExternalInput")
    iota2 = nc.dram_tensor("iota2", [TPE, 8 * BLK2], BF16, kind="ExternalInput")
    iotas = nc.dram_tensor("iotas", [TPE, 2 * GRP], BF16, kind="ExternalInput")
    ident = nc.dram_tensor("ident", [TPE, TPE], BF16, kind="ExternalInput")
    bas1 = nc.dram_tensor("bas1", [D, 4 * D], BF16, kind="ExternalInput")
    bas2 = nc.dram_tensor("bas2", [D, 4 * D], BF16, kind="ExternalInput")
    rt1 = nc.dram_tensor("rt1", [D, D], BF16, kind="ExternalInput")
    rt2 = nc.dram_tensor("rt2", [D, D], BF16, kind="ExternalInput")
    bias1 = nc.dram_tensor("bias1", [GRP, D], F32, kind="ExternalInput")
    bias2 = nc.dram_tensor("bias2", [GRP, D], F32, kind="ExternalInput")
    outp = nc.dram_tensor("outp", [NPC, D], F32, kind="ExternalOutput")

    with tile.TileContext(nc) as tc:
        with (
            tc.tile_pool(name="const", bufs=1) as cst,
            tc.tile_pool(name="meta", bufs=3) as meta,
            tc.tile_pool(name="gath", bufs=3) as gath,
            tc.tile_pool(name="woh", bufs=3) as wohp,
            tc.tile_pool(name="hp", bufs=3) as hp,
            tc.tile_pool(name="xtp", bufs=3) as xtp,
            tc.tile_pool(name="sbig", bufs=2) as sbigp,
            tc.tile_pool(name="ps", bufs=1, space="PSUM") as psp,
            tc.tile_pool(name="dram", bufs=1, space="DRAM") as dramp,
        ):
            pools = (meta, gath, wohp, hp, xtp, sbigp, psp)

            iota_t = cst.tile([TPE, CHK * BLK], BF16)
            nc.sync.dma_start(out=iota_t[:], in_=iota[:])
            iota2_t = cst.tile([TPE, 8 * BLK2], BF16)
            nc.sync.dma_start(out=iota2_t[:], in_=iota2[:])
            iotas_t = cst.tile([TPE, 2 * GRP], BF16)
            nc.sync.dma_start(out=iotas_t[:], in_=iotas[:])
            xslb = cst.tile([D, NGRP, GRP], BF16)
            nc.sync.dma_start(out=xslb[:], in_=xslp[:])
            h4buf = cst.tile([GRP, NGRP, DW], BF16)
            nc.vector.memset(h4buf[:], 0.0)
            ident_t = cst.tile([TPE, TPE], BF16)
            nc.sync.dma_start(out=ident_t[:], in_=ident[:])
            bas1_t = cst.tile([D, 4 * D], BF16)
            nc.sync.dma_start(out=bas1_t[:], in_=bas1[:])
            bas2_t = cst.tile([D, 4 * D], BF16)
            nc.sync.dma_start(out=bas2_t[:], in_=bas2[:])
            rt1_t = cst.tile([D, D], BF16)
            nc.sync.dma_start(out=rt1_t[:], in_=rt1[:])
            rt2_t = cst.tile([D, D], BF16)
            nc.sync.dma_start(out=rt2_t[:], in_=rt2[:])
            bias1_t = cst.tile([GRP, D], F32)
            nc.sync.dma_start(out=bias1_t[:], in_=bias1[:])
            bias2_t = cst.tile([GRP, D], F32)
            nc.sync.dma_start(out=bias2_t[:], in_=bias2[:])
            zed_t = cst.tile([GRP, 2 * DW], BF16)
            nc.gpsimd.memset(zed_t[:], 0.0)

            hsl_cs = [dramp.tile([pr_c[c], 2 * DW], BF16, name=f"hsl_c{c}")
                      for c in range(NCHUNK)]
            hfull = dramp.tile([NTAB // 2, 2 * DW], BF16)

            # zero rows at the tail of chunk 0 (dummy gather targets)
            nc.sync.dma_start(
                out=hsl_cs[0][pr_c[0] - GRP // 2:pr_c[0], :],
                in_=zed_t[0:GRP // 2, :])

            fired = [False] * NCHUNK
            last_group_of_chunk = np.cumsum(cg) - 1

            def group_done(g):
                for c in range(NCHUNK):
                    if g == last_group_of_chunk[c] and not fired[c]:
                        fired[c] = True
                        base = int(prm["hfull_base"][c] // 1)
                        rows = pr_c[c] * M
                        nc.gpsimd.collective_compute(
                            "AllGather",
                            ALU.bypass,
                            replica_groups=[list(range(M))],
                            ins=[hsl_cs[c][:]],
                            outs=[hfull[base:base + rows, :]],
                        )

            _layer1(tc, nc, pools, prm, D, xpg, metac1,
                    iota_t[:], ident_t[:], bas1_t, rt1_t[:],
                    bias1_t[:], xslb, h4buf, hsl_cs, group_done)
            _layer2(tc, nc, pools, prm, D, hfull[:, :], idxp, metac2,
                    iota2_t[:], iotas_t[:], ident_t[:], bas2_t,
                    rt2_t[:], bias2_t[:], h4buf, outp)
    nc.compile()
    return nc


def kernel(entity, edge_index, edge_attr, edge_type, edge_norm,
           basis1, att1, root1, bias1, basis2, att2, root2, bias2):
    N, D = entity.shape
    prm = _prep(N, D, np.asarray(entity), np.asarray(edge_index),
                np.asarray(edge_type), np.asarray(edge_norm),
                np.asarray(att1), np.asarray(att2))
    NPC = prm["NPC"]

    entity = np.asarray(entity, dtype=np.float32)

    iota_arr = np.tile(np.arange(BLK, dtype=np.float32), (TPE, CHK)).astype(BF)
    iota2_arr = np.tile(np.arange(BLK2, dtype=np.float32), (TPE, 8)).astype(BF)
    iotas_arr = np.tile(np.arange(2 * GRP, dtype=np.float32), (TPE, 1)).astype(BF)
    ident_arr = np.eye(TPE, dtype=np.float32).astype(BF)
    b1 = np.ascontiguousarray(
        np.asarray(basis1, np.float32).transpose(1, 0, 2).reshape(D, 4 * D)).astype(BF)
    b2 = np.ascontiguousarray(
        np.asarray(basis2, np.float32).transpose(1, 0, 2).reshape(D, 4 * D)).astype(BF)

    nc = _build(prm, D)

    in_maps = []
    for m in range(M):
        lo, hi = m * NPC, min((m + 1) * NPC, N)
        xs = np.zeros((NPC, D), dtype=np.float32)
        if hi > lo:
            xs[0:hi - lo] = entity[lo:hi]
        xslp_arr = np.ascontiguousarray(
            xs.reshape(prm["NGRP"], GRP, D).transpose(2, 0, 1)
            .reshape(D, prm["NGRP"] * GRP)).astype(BF)
        in_maps.append({
            "xslp": xslp_arr,
            "xpg": prm["xpg"][m],
            "idxp": prm["idxp"][m],
            "metac1": prm["metac1"][m],
            "metac2": prm["metac2"][m],
            "iota": iota_arr,
            "iota2": iota2_arr,
            "iotas": iotas_arr,
            "ident": ident_arr,
            "bas1": b1,
            "bas2": b2,
            "rt1": np.asarray(root1, np.float32).astype(BF),
            "rt2": np.asarray(root2, np.float32).astype(BF),
            "bias1": np.tile(np.asarray(bias1, np.float32), (GRP, 1)),
            "bias2": np.tile(np.asarray(bias2, np.float32), (GRP, 1)),
        })
    kwargs = {}
    if TRACE:
        kwargs = dict(trace=True, tmpdir=TRACE_DIR)
    res = run_bass_kernel_spmd(nc, in_maps, core_ids=list(range(M)), **kwargs)
    global LAST
    LAST = res
    out = np.concatenate([res.results[m]["outp"] for m in range(M)], axis=0)
    return np.ascontiguousarray(out[:N])


LAST = None
TRACE = False
TRACE_DIR = None



# revision 3
# speedup vs baseline: 1.0691x; 1.0691x over previous
"""RGCN (2-layer, basis-decomposition) Trainium2 kernel, v2.

Strategy (8 NeuronCores, SPMD), building on v1:
  - Edges sorted by destination; dst nodes partitioned into 8 contiguous
    ranges (one per core). Messages + segment-sum fused into per-tile PE
    matmuls against a DVE-built "weighted one-hot" (woh) matrix.
  - NEW in v2: layer 1's per-edge source rows are pre-gathered ON THE HOST
    into tile-slot order (the gather indices are static), so layer 1 does
    dense sequential DMA loads instead of gpsimd dma_gather (which costs
    ~8 ns/row of Q7 descriptor generation). This removes the parity trick
    for layer 1: one matmul per tile, 4 instead of 8 woh multiplies.
  - NEW in v2: the inter-layer AllGather of h is split into 4 chunks, each
    fired as soon as its groups' h rows are written -- overlapping the
    collective with layer-1 tail compute (gpsimd is idle during layer 1).
  - Layer 2 keeps the dma_gather path (h is device-computed): bf16 rows
    paired into 512-B table rows to fit int16 gather indices; even/odd
    parity handled by two matmuls per tile against parity-masked wohs.
"""

import math

import numpy as np
import ml_dtypes

import concourse.bacc as bacc
import concourse.bass as bass
import concourse.mybir as mybir
import concourse.tile as tile
from concourse.bass_utils import run_bass_kernel_spmd

F32 = mybir.dt.float32
BF16 = mybir.dt.bfloat16
I16 = mybir.dt.int16
AF = mybir.ActivationFunctionType
ALU = mybir.AluOpType
BF = ml_dtypes.bfloat16

M = 8            # cores
BLK = 32         # nodes per scatter block
GRP = 128        # nodes per output group (4 blocks)
TPE = 128        # edges per tile
G = 32           # tiles per gather page
CHK = 8          # tiles per wonehot build chunk
DW = 128         # padded table row width (256 B in bf16)
BLK2 = 64        # layer-2 block size (fewer tiles -> fewer gather descriptors)
CAPW = 4         # base tiles per BLK2 block (overflow goes to spill tiles)
NCHUNK = 4       # AllGather chunks


def _expand(ap, free_dims, col_offset=0):
    """AP with the partition dim kept and explicit [step, count] free dims."""
    base = ap.ap
    return bass.AP(
        ap.tensor,
        ap.offset + col_offset,
        [list(base[0])] + [list(d) for d in free_dims],
    )


def _prep(N, D, entity, edge_index, edge_type, edge_norm, att1, att2):
    """Host-side graph preprocessing. Returns per-core arrays + structure."""
    NPC = int(math.ceil(N / (M * GRP))) * GRP      # nodes per core (6272)
    SEG = NPC + GRP                                # table segment per core (6400)
    NTAB = SEG * M                                 # 51200
    NBLK = NPC // BLK
    NGRP = NPC // GRP

    # AllGather chunking of the 49 groups: sizes ~equal; last chunk also
    # holds the zero-row region (GRP extra nodes).
    base_cg = NGRP // NCHUNK
    cg = [base_cg] * NCHUNK
    cg[0] += NGRP - base_cg * NCHUNK               # [13,12,12,12]
    chunk_of_group = np.repeat(np.arange(NCHUNK), cg)
    chunk_gstart = np.concatenate([[0], np.cumsum(cg)[:-1]])
    pr_c = [x * (GRP // 2) for x in cg]
    pr_c[0] += GRP // 2                            # zero region pair rows
    hfull_base = np.concatenate([[0], np.cumsum([p * M for p in pr_c])[:-1]])
    zero_row = int(hfull_base[0] + pr_c[0] - GRP // 2)  # core 0's zero region

    src = np.asarray(edge_index[0], dtype=np.int64)
    dst = np.asarray(edge_index[1], dtype=np.int64)
    et = np.asarray(edge_type[:, 0], dtype=np.int64)
    norm = np.asarray(edge_norm, dtype=np.float32)

    order = np.argsort(dst, kind="stable")
    src_s, dst_s, et_s, norm_s = src[order], dst[order], et[order], norm[order]

    deg = np.bincount(dst, minlength=NPC * M).astype(np.float32)
    inv_deg = (1.0 / np.maximum(deg, 1.0)).astype(np.float32)

    def tilestruct(BLKx):
        """Per-layer tile structure for block size BLKx."""
        nblk = NPC // BLKx
        gb = dst_s // BLKx
        cnt = np.bincount(gb, minlength=nblk * M).reshape(M, nblk)
        T_k = np.maximum(1, -(-cnt // TPE)).max(axis=0)
        T_total = int(T_k.sum())
        npage = -(-T_total // G)
        T_k[-1] += npage * G - T_total
        T = npage * G
        tb = np.zeros(nblk, dtype=np.int64)
        tb[1:] = np.cumsum(T_k)[:-1]
        lb = gb % nblk
        es = np.zeros(nblk * M + 1, dtype=np.int64)
        es[1:] = np.cumsum(np.bincount(gb, minlength=nblk * M))
        within = np.arange(len(dst_s)) - es[gb]
        slot = tb[lb] * TPE + within
        tiles = []
        for k in range(nblk):
            for t in range(int(T_k[k])):
                tiles.append((k, t == 0, t == int(T_k[k]) - 1))
        return dict(slot=slot, T=T, NPAGE=npage, tiles=tiles,
                    doff=(dst_s % BLKx).astype(np.float32))

    st1 = tilestruct(BLK)
    core_of = dst_s // NPC

    # ---- layer-2 spill structure: per block-pair (BLK2) 4 base tiles,
    # plus one group-wide spill tile per group; spans of 2 groups ----
    CAP = CAPW                    # base tiles per BLK2 block
    G2 = 34                       # tiles per layer-2 page (2 spans of 17)
    nblk2 = NPC // BLK2
    # chunk/tile schedule (shared by host packing and kernel build)
    l2_tiles = []                 # per tile: (kind, g, bi, w)
    l2_chunks = []                # per chunk: (kind, CHKs, tile_off_global)
    for j in range((NGRP + 1) // 2):
        gs = [2 * j] + ([2 * j + 1] if 2 * j + 1 < NGRP else [])
        l2_chunks.append(("s", 1, len(l2_tiles)))
        l2_tiles.append(("s", j, len(gs), 0))
        for half in range(len(gs)):
            g = gs[half]
            l2_chunks.append(("b", 8, len(l2_tiles)))
            for bi in range(2):
                for w in range(CAP):
                    l2_tiles.append(("b", g, bi, w))
    T2 = len(l2_tiles)            # 441
    NPAGE2 = -(-T2 // G2)         # 13
    l2_page_tiles = [min(G2, T2 - p * G2) for p in range(NPAGE2)]
    # base tile slot ranges: tile index of (block k, w)
    base_tile_id = np.zeros((nblk2, CAP), dtype=np.int64)
    spill_tile_id = np.zeros(NGRP, dtype=np.int64)
    for t, (kind, g, bi, w) in enumerate(l2_tiles):
        if kind == "s":
            spill_tile_id[2 * g] = t
            if 2 * g + 1 < NGRP:
                spill_tile_id[2 * g + 1] = t
        else:
            base_tile_id[2 * g + bi, w] = t

    nd = norm_s * inv_deg[dst_s]
    c1 = (att1[et_s] * nd[:, None]).astype(np.float32)
    c2 = (att2[et_s] * nd[:, None]).astype(np.float32)

    # layer-2 gather: chunk-major hfull pair-row index for each source node
    k_s = src_s // NPC
    n_s = src_s % NPC
    g_s = n_s // GRP
    c_s = chunk_of_group[g_s]
    lpr = (g_s - chunk_gstart[c_s]) * (GRP // 2) + (n_s % GRP) // 2
    row_s = hfull_base[c_s] + k_s * np.asarray(pr_c)[c_s] + lpr
    par_s = (n_s % 2).astype(np.float32)

    def pack_idx(lin, npage):
        """[npage*G2*TPE] indices -> [npage, 128, G2*8] int16 gather layout."""
        out = np.empty((npage, 16, G2 * 8), np.int16)
        lp = lin.reshape(npage, G2 * TPE)
        out[:, :, :] = lp.reshape(npage, G2 * 8, 16).transpose(0, 2, 1)
        return np.ascontiguousarray(np.tile(out, (1, 8, 1)))

    def scat(vals, sl, T, fill=0.0, width=None):
        if width is None:
            out = np.full(T * TPE, fill, dtype=np.float32)
        else:
            out = np.full((T * TPE, width), fill, dtype=np.float32)
        out[sl] = vals
        return out

    def tilemaj(a, npage, w, Gx=G):
        """[T*TPE(,w)] slot-major -> [npage, TPE, Gx*w] page/tile-major bf16."""
        if a.ndim == 1:
            a = a[:, None]
        return np.ascontiguousarray(
            a.reshape(npage, Gx, TPE, a.shape[1]).transpose(0, 2, 1, 3)
            .reshape(npage, TPE, Gx * a.shape[1]).astype(BF))

    ent = np.asarray(entity, dtype=np.float32)

    metac1s, metac2s, idx_pages, xpgs = [], [], [], []
    for m in range(M):
        sel = core_of == m
        sl1 = st1["slot"][sel]
        T1, NP1 = st1["T"], st1["NPAGE"]

        # layer-1 pre-gathered source rows, page layout [NP1, 128, G*D]
        xsrcm = np.zeros((T1 * TPE, D), dtype=BF)
        xsrcm[sl1] = ent[src_s[sel]].astype(BF)
        xpgs.append(tilemaj(xsrcm.astype(np.float32), NP1, D))

        # layer 1 meta: no parity; cols [0:G] doff, [G:5G] coeffs
        metac1 = np.concatenate([
            tilemaj(scat(st1["doff"][sel], sl1, T1, 99.0), NP1, 1),
            tilemaj(scat(c1[sel], sl1, T1, width=4), NP1, 4)], axis=2)
        metac1s.append(np.ascontiguousarray(metac1))

        # layer-2 slot assignment: base tiles per block (cap CAP*TPE edges),
        # overflow into the group's spill tile
        eidx = np.nonzero(sel)[0]
        gb2 = dst_s[eidx] // BLK2
        es2 = np.zeros(nblk2 + 1, dtype=np.int64)
        es2[1:] = np.cumsum(np.bincount(gb2 - m * nblk2, minlength=nblk2))
        w2 = np.arange(len(eidx)) - es2[gb2 - m * nblk2]
        lb2 = gb2 - m * nblk2
        is_base = w2 < CAP * TPE
        sl2 = np.empty(len(eidx), dtype=np.int64)
        sl2[is_base] = (base_tile_id[lb2[is_base], w2[is_base] // TPE] * TPE
                        + w2[is_base] % TPE)
        spill_e = ~is_base
        sg = lb2[spill_e] // 2
        # spill slots shared per span (4 blocks): offset each block's spill
        # edges by the spill counts of preceding blocks in its span
        sp_cnt = np.maximum(0, np.diff(es2) - CAP * TPE)
        tot = np.cumsum(sp_cnt) - sp_cnt
        span_of_blk = np.arange(nblk2) // 4
        first = np.searchsorted(span_of_blk, np.arange(span_of_blk[-1] + 1))
        sp_base = tot - tot[first][span_of_blk]
        sw = w2[spill_e] - CAP * TPE + sp_base[lb2[spill_e]]
        assert sw.max(initial=0) < TPE, sw.max(initial=0)
        sl2[spill_e] = spill_tile_id[sg] * TPE + sw
        TSLOTS = NPAGE2 * G2
        rows = np.full(TSLOTS * TPE, zero_row, dtype=np.int64)
        rows[sl2] = row_s[eidx]
        idx_pages.append(pack_idx(rows.astype(np.int16), NPAGE2))
        par = scat(par_s[eidx], sl2, TSLOTS)

        do2 = np.where(is_base, dst_s[eidx] % BLK2,
                       (dst_s[eidx] % NPC) % (2 * GRP))
        ca = scat(c2[eidx], sl2, TSLOTS, width=4)
        metac2 = np.concatenate([
            tilemaj(scat(do2.astype(np.float32), sl2, TSLOTS, 320.0), NPAGE2, 1, G2),
            tilemaj(ca * (1.0 - par)[:, None], NPAGE2, 4, G2),
            tilemaj(ca * par[:, None], NPAGE2, 4, G2)], axis=2)
        metac2s.append(np.ascontiguousarray(metac2))


    return dict(NPC=NPC, SEG=SEG, NTAB=NTAB, NGRP=NGRP,
                NPAGE1=st1["NPAGE"], NPAGE2=NPAGE2, G2=G2,
                tiles1=st1["tiles"], l2_tiles=l2_tiles, l2_chunks=l2_chunks,
                l2_page_tiles=l2_page_tiles,
                cg=cg, chunk_of_group=chunk_of_group,
                chunk_gstart=chunk_gstart, pr_c=pr_c, hfull_base=hfull_base,
                idxp=idx_pages, metac1=metac1s, metac2=metac2s, xpg=xpgs)


TAIL_DELAY = 5   # chunks between a group's last matmul and its tail emission
TAIL_DELAY2 = 2  # layer-2 variant (faster wind-down at the end)


def _layer1(tc, nc, pools, prm, D, xpg, metap, iota_t, ident_t,
            bas_t, rt_t, bias_t, xslb, h4buf, hsl_cs, group_done):
    """Dense layer 1: pre-gathered x rows, single matmul per tile."""
    meta, gath, wohp, hp, xtp, sbigp, psp = pools
    NPAGE, tile_blocks = prm["NPAGE1"], prm["tiles1"]

    pending = []

    def flush(now):
        while pending and (now is None or pending[0][0] + TAIL_DELAY <= now):
            _, g, sbig = pending.pop(0)
            _tail1(tc, nc, pools, prm, D, g, sbig, xslb, h4buf,
                   ident_t, bas_t, rt_t, bias_t, hsl_cs)
            group_done(g)

    tcount = 0
    psum_blk = None
    for q in range(NPAGE):
        metat = meta.tile([TPE, 5 * G], BF16, tag="metat1")
        nc.sync.dma_start(out=metat[:], in_=metap[q])
        gbuf = gath.tile([TPE, G * D], BF16, tag="gbuf1", bufs=4)
        nc.sync.dma_start(out=gbuf[:], in_=xpg[q])
        for c in range(G // CHK):
            cglob = q * (G // CHK) + c
            flush(cglob)
            oh8 = wohp.tile([TPE, CHK * BLK], BF16, tag="oh1", bufs=5)
            nc.vector.tensor_tensor(
                out=_expand(oh8[:], [[BLK, CHK], [1, BLK]]),
                in0=iota_t,
                in1=_expand(metat[:], [[1, CHK], [0, BLK]],
                            col_offset=c * CHK),
                op=ALU.is_equal,
            )
            woh = wohp.tile([TPE, CHK * 4 * BLK], BF16, tag="woh1", bufs=5)
            for b in range(4):
                nc.vector.tensor_tensor(
                    out=_expand(woh[:], [[4 * BLK, CHK], [1, BLK]],
                                col_offset=b * BLK),
                    in0=_expand(oh8[:], [[BLK, CHK], [1, BLK]]),
                    in1=_expand(metat[:], [[4, CHK], [0, BLK]],
                                col_offset=G + c * CHK * 4 + b),
                    op=ALU.mult,
                )
            for u in range(CHK):
                blk, first, last = tile_blocks[tcount]
                g, bi = blk // 4, blk % 4
                if first and bi == 0:
                    psum_blk = psp.tile([D, 4 * GRP], F32, tag="blk", bufs=3)
                out_sl = psum_blk[:, bi * 4 * BLK:(bi + 1) * 4 * BLK]
                nc.tensor.matmul(out=out_sl,
                                 lhsT=gbuf[:, (c * CHK + u) * D:(c * CHK + u + 1) * D],
                                 rhs=woh[:, u * 4 * BLK:(u + 1) * 4 * BLK],
                                 start=first, stop=last)
                if last and bi == 3:
                    sbig = sbigp.tile([D, 4 * GRP], BF16, tag="sbig", bufs=6)
                    for b in range(4):
                        nc.scalar.copy(
                            out=_expand(sbig[:], [[BLK, 4], [1, BLK]],
                                        col_offset=b * GRP),
                            in_=_expand(psum_blk[:], [[4 * BLK, 4], [1, BLK]],
                                        col_offset=b * BLK),
                        )
                    pending.append((cglob, g, sbig))
                tcount += 1
    flush(None)


def _tail1(tc, nc, pools, prm, D, g, sbig, xslb, h4buf, ident_t,
           bas_t, rt_t, bias_t, hsl_cs):
    """Layer-1 group tail: combine bases, mean, root, bias, relu -> hsl chunk."""
    meta, gath, wohp, hp, xtp, sbigp, psp = pools
    pagg = psp.tile([GRP, D], F32, tag="agg", bufs=2)
    for b in range(4):
        nc.tensor.matmul(
            out=pagg[:],
            lhsT=sbig[:, b * GRP:(b + 1) * GRP],
            rhs=bas_t[:, b * D:(b + 1) * D],
            start=(b == 0),
            stop=False,
        )
    nc.tensor.matmul(out=pagg[:], lhsT=xslb[:, g, :], rhs=rt_t,
                     start=False, stop=True)

    h3 = hp.tile([GRP, D], F32, tag="h3")
    nc.vector.tensor_tensor(out=h3[:], in0=pagg[:], in1=bias_t, op=ALU.add)

    nc.scalar.activation(out=h4buf[:, g, 0:D], in_=h3[:], func=AF.Relu)
    c = int(prm["chunk_of_group"][g])
    gl = g - int(prm["chunk_gstart"][c])
    base = hsl_cs[c][:]
    dst = bass.AP(base.tensor, base.offset + gl * (GRP // 2) * (2 * DW),
                  [[2 * DW, GRP // 2], [DW, 2], [1, DW]])
    nc.sync.dma_start(out=dst, in_=h4buf[:, g, :])


def _layer2(tc, nc, pools, prm, D, table_ap, idxp, metap,
            iota_t, iota_s, ident_t, bas_t, rt_t, bias_t, h4buf,
            out_ap):
    """Gather-based layer 2: per-block base tiles + per-group spill tiles."""
    meta, gath, wohp, hp, xtp, sbigp, psp = pools
    NPAGE, G2 = prm["NPAGE2"], prm["G2"]
    l2_tiles, l2_chunks = prm["l2_tiles"], prm["l2_chunks"]
    page_tiles = prm["l2_page_tiles"]

    pending = []

    def flush(now):
        while pending and (now is None or pending[0][0] + TAIL_DELAY2 <= now):
            _, g, sbig = pending.pop(0)
            _tail2(tc, nc, pools, prm, D, g, sbig, h4buf, ident_t,
                   bas_t, rt_t, bias_t, out_ap)

    # split chunks by page (a chunk never crosses a page boundary:
    # pages are 36 tiles = 2 spans of 18, chunks are 2+8+8 per span)
    chunks_by_page = [[] for _ in range(NPAGE)]
    for kind, chks, toff in l2_chunks:
        chunks_by_page[toff // G2].append((kind, chks, toff))

    psums = {}
    cglob = 0
    for q in range(NPAGE):
        nt = page_tiles[q]
        nidx = nt * TPE
        idxt = meta.tile([TPE, 8 * G2], I16, tag="idxt", bufs=4)
        nc.sync.dma_start(out=idxt[:], in_=idxp[q])
        metat = meta.tile([TPE, 9 * G2], BF16, tag="metat2", bufs=4)
        nc.scalar.dma_start(out=metat[:], in_=metap[q])
        gbuf = gath.tile([TPE, G2, 2 * DW], BF16, tag="gbuf2", bufs=4)
        nc.gpsimd.dma_gather(
            out_ap=gbuf[:, 0:nt, :], in_ap=table_ap,
            idxs_ap=idxt[:, 0:nidx // 16], num_idxs=nidx, num_idxs_reg=nidx,
            elem_size=2 * DW, single_packet=False, queue_num=q % 4)
        for kind, chks, toff in chunks_by_page[q]:
            flush(cglob)
            cglob += 1
            loff = toff - q * G2
            BLKx = 2 * GRP if kind == "s" else BLK2
            iot = iota_s if kind == "s" else iota_t
            oh8 = wohp.tile([TPE, 8 * BLK2], BF16, tag="oh2", bufs=7)
            nc.vector.tensor_tensor(
                out=_expand(oh8[:], [[BLKx, chks], [1, BLKx]]),
                in0=iot[:, 0:chks * BLKx],
                in1=_expand(metat[:], [[1, chks], [0, BLKx]],
                            col_offset=loff),
                op=ALU.is_equal,
            )
            wohE = wohp.tile([TPE, 8 * 4 * BLK2], BF16, tag="wohE", bufs=6)
            wohO = wohp.tile([TPE, 8 * 4 * BLK2], BF16, tag="wohO", bufs=6)
            for woh, cbase in ((wohE, G2), (wohO, 5 * G2)):
                for b in range(4):
                    if kind == "s":
                        # spill (span): col ghalf*512 + bi*256 + b*64 + j%64,
                        # matching the base tiles' psum layout
                        nc.vector.tensor_tensor(
                            out=_expand(woh[:],
                                        [[8 * BLK2, 2], [4 * BLK2, 2],
                                         [1, BLK2]],
                                        col_offset=b * BLK2),
                            in0=_expand(oh8[:],
                                        [[2 * BLK2, 2], [BLK2, 2], [1, BLK2]]),
                            in1=_expand(metat[:],
                                        [[0, 2], [0, 2], [0, BLK2]],
                                        col_offset=cbase + loff * 4 + b),
                            op=ALU.mult,
                        )
                    else:
                        nc.vector.tensor_tensor(
                            out=_expand(woh[:], [[4 * BLKx, chks], [1, BLKx]],
                                        col_offset=b * BLKx),
                            in0=_expand(oh8[:], [[BLKx, chks], [1, BLKx]]),
                            in1=_expand(metat[:], [[4, chks], [0, BLKx]],
                                        col_offset=cbase + loff * 4 + b),
                            op=ALU.mult,
                        )
            for u in range(chks):
                kindt, g, bi, w = l2_tiles[toff + u]
                lt = loff + u
                if kindt == "s":
                    for gh in range(bi):       # bi = ngroups in this span
                        gg = 2 * g + gh
                        psums[gg] = psp.tile([D, 4 * GRP], F32, tag="blk",
                                             bufs=3, name="psum_blk")
                        nc.tensor.matmul(
                            out=psums[gg][:, :], lhsT=gbuf[:, lt, 0:D],
                            rhs=wohE[:, gh * 512:(gh + 1) * 512],
                            start=True, stop=False)
                        nc.tensor.matmul(
                            out=psums[gg][:, :], lhsT=gbuf[:, lt, DW:DW + D],
                            rhs=wohO[:, gh * 512:(gh + 1) * 512],
                            start=False, stop=False)
                    continue
                out_sl = psums[g][:, bi * 4 * BLK2:(bi + 1) * 4 * BLK2]
                first, stop = False, (w == CAPW - 1)
                nc.tensor.matmul(out=out_sl, lhsT=gbuf[:, lt, 0:D],
                                 rhs=wohE[:, u * 4 * BLKx:(u + 1) * 4 * BLKx],
                                 start=first, stop=False)
                nc.tensor.matmul(out=out_sl,
                                 lhsT=gbuf[:, lt, DW:DW + D],
                                 rhs=wohO[:, u * 4 * BLKx:(u + 1) * 4 * BLKx],
                                 start=False, stop=stop)
                if kindt == "b" and bi == 1 and w == CAPW - 1:
                    psum_blk = psums.pop(g)
                    sbig = sbigp.tile([D, 4 * GRP], BF16, tag="sbig", bufs=6)
                    for b in range(4):
                        nc.scalar.copy(
                            out=_expand(sbig[:], [[BLK2, 2], [1, BLK2]],
                                        col_offset=b * GRP),
                            in_=_expand(psum_blk[:], [[4 * BLK2, 2], [1, BLK2]],
                                        col_offset=b * BLK2),
                        )
                    pending.append((cglob, g, sbig))
    flush(None)


def _tail2(tc, nc, pools, prm, D, g, sbig, h4buf, ident_t,
           bas_t, rt_t, bias_t, out_ap):
    meta, gath, wohp, hp, xtp, sbigp, psp = pools
    ptr = psp.tile([D, GRP], BF16, tag="tr", bufs=2)
    nc.tensor.transpose(out=ptr[:], in_=h4buf[:, g, 0:D], identity=ident_t)
    xt = xtp.tile([D, GRP], BF16, tag="xt")
    nc.scalar.copy(out=xt[:], in_=ptr[:])

    pagg = psp.tile([GRP, D], F32, tag="agg", bufs=2)
    for b in range(4):
        nc.tensor.matmul(
            out=pagg[:],
            lhsT=sbig[:, b * GRP:(b + 1) * GRP],
            rhs=bas_t[:, b * D:(b + 1) * D],
            start=(b == 0),
            stop=False,
        )
    nc.tensor.matmul(out=pagg[:], lhsT=xt[:], rhs=rt_t, start=False, stop=True)

    h3 = hp.tile([GRP, D], F32, tag="h3")
    nc.vector.tensor_tensor(out=h3[:], in0=pagg[:], in1=bias_t, op=ALU.add)
    nc.scalar.dma_start(out=out_ap[g * GRP:(g + 1) * GRP, :], in_=h3[:])


def _build(prm, D):
    NPC, NTAB = prm["NPC"], prm["NTAB"]
    NPAGE1, NPAGE2, NGRP = prm["NPAGE1"], prm["NPAGE2"], prm["NGRP"]
    pr_c, cg = prm["pr_c"], prm["cg"]
    nc = bacc.Bacc(num_swdge_queues=4)

    xslp = nc.dram_tensor("xslp", [D, NGRP * GRP], BF16, kind="ExternalInput")
    xpg = nc.dram_tensor("xpg", [NPAGE1, TPE, G * D], BF16, kind="ExternalInput")
    G2 = prm["G2"]
    idxp = nc.dram_tensor("idxp", [NPAGE2, TPE, 8 * G2], I16, kind="ExternalInput")
    metac1 = nc.dram_tensor("metac1", [NPAGE1, TPE, 5 * G], BF16, kind="ExternalInput")
    metac2 = nc.dram_tensor("metac2", [NPAGE2, TPE, 9 * G2], BF16, kind="ExternalInput")
    iota = nc.dram_tensor("iota", [TPE, CHK * BLK], BF16, kind="ExternalInput")
    iota2 = nc.dram_tensor("iota2", [TPE, 8 * BLK2], BF16, kind="ExternalInput")
    iotas = nc.dram_tensor("iotas", [TPE, 2 * GRP], BF16, kind="ExternalInput")
    ident = nc.dram_tensor("ident", [TPE, TPE], BF16, kind="ExternalInput")
    bas1 = nc.dram_tensor("bas1", [D, 4 * D], BF16, kind="ExternalInput")
    bas2 = nc.dram_tensor("bas2", [D, 4 * D], BF16, kind="ExternalInput")
    rt1 = nc.dram_tensor("rt1", [D, D], BF16, kind="ExternalInput")
    rt2 = nc.dram_tensor("rt2", [D, D], BF16, kind="ExternalInput")
    bias1 = nc.dram_tensor("bias1", [GRP, D], F32, kind="ExternalInput")
    bias2 = nc.dram_tensor("bias2", [GRP, D], F32, kind="ExternalInput")
    outp = nc.dram_tensor("outp", [NPC, D], F32, kind="ExternalOutput")

    with tile.TileContext(nc) as tc:
        with (
            tc.tile_pool(name="const", bufs=1) as cst,
            tc.tile_pool(name="meta", bufs=3) as meta,
            tc.tile_pool(name="gath", bufs=3) as gath,
            tc.tile_pool(name="woh", bufs=3) as wohp,
            tc.tile_pool(name="hp", bufs=3) as hp,
            tc.tile_pool(name="xtp", bufs=3) as xtp,
            tc.tile_pool(name="sbig", bufs=2) as sbigp,
            tc.tile_pool(name="ps", bufs=1, space="PSUM") as psp,
            tc.tile_pool(name="dram", bufs=1, space="DRAM") as dramp,
        ):
            pools = (meta, gath, wohp, hp, xtp, sbigp, psp)

            iota_t = cst.tile([TPE, CHK * BLK], BF16)
            nc.sync.dma_start(out=iota_t[:], in_=iota[:])
            iota2_t = cst.tile([TPE, 8 * BLK2], BF16)
            nc.sync.dma_start(out=iota2_t[:], in_=iota2[:])
            iotas_t = cst.tile([TPE, 2 * GRP], BF16)
            nc.sync.dma_start(out=iotas_t[:], in_=iotas[:])
            xslb = cst.tile([D, NGRP, GRP], BF16)
            nc.sync.dma_start(out=xslb[:], in_=xslp[:])
            h4buf = cst.tile([GRP, NGRP, DW], BF16)
            nc.vector.memset(h4buf[:], 0.0)
            ident_t = cst.tile([TPE, TPE], BF16)
            nc.sync.dma_start(out=ident_t[:], in_=ident[:])
            bas1_t = cst.tile([D, 4 * D], BF16)
            nc.sync.dma_start(out=bas1_t[:], in_=bas1[:])
            bas2_t = cst.tile([D, 4 * D], BF16)
            nc.sync.dma_start(out=bas2_t[:], in_=bas2[:])
            rt1_t = cst.tile([D, D], BF16)
            nc.sync.dma_start(out=rt1_t[:], in_=rt1[:])
            rt2_t = cst.tile([D, D], BF16)
            nc.sync.dma_start(out=rt2_t[:], in_=rt2[:])
            bias1_t = cst.tile([GRP, D], F32)
            nc.sync.dma_start(out=bias1_t[:], in_=bias1[:])
            bias2_t = cst.tile([GRP, D], F32)
            nc.sync.dma_start(out=bias2_t[:], in_=bias2[:])
            zed_t = cst.tile([GRP, 2 * DW], BF16)
            nc.gpsimd.memset(zed_t[:], 0.0)

            hsl_cs = [dramp.tile([pr_c[c], 2 * DW], BF16, name=f"hsl_c{c}")
                      for c in range(NCHUNK)]
            hfull = dramp.tile([NTAB // 2, 2 * DW], BF16)

            # zero rows at the tail of chunk 0 (dummy gather targets)
            nc.sync.dma_start(
                out=hsl_cs[0][pr_c[0] - GRP // 2:pr_c[0], :],
                in_=zed_t[0:GRP // 2, :])

            fired = [False] * NCHUNK
            last_group_of_chunk = np.cumsum(cg) - 1

            def group_done(g):
                for c in range(NCHUNK):
                    if g == last_group_of_chunk[c] and not fired[c]:
                        fired[c] = True
                        base = int(prm["hfull_base"][c] // 1)
                        rows = pr_c[c] * M
                        nc.gpsimd.collective_compute(
                            "AllGather",
                            ALU.bypass,
                            replica_groups=[list(range(M))],
                            ins=[hsl_cs[c][:]],
                            outs=[hfull[base:base + rows, :]],
                        )

            _layer1(tc, nc, pools, prm, D, xpg, metac1,
                    iota_t[:], ident_t[:], bas1_t, rt1_t[:],
                    bias1_t[:], xslb, h4buf, hsl_cs, group_done)
            _layer2(tc, nc, pools, prm, D, hfull[:, :], idxp, metac2,
                    iota2_t[:], iotas_t[:], ident_t[:], bas2_t,
                    rt2_t[:], bias2_t[:], h4buf, outp)
    nc.compile()
    return nc


def kernel(entity, edge_index, edge_attr, edge_type, edge_norm,
           basis1, att1, root1, bias1, basis2, att2, root2, bias2):
    N, D = entity.shape
    prm = _prep(N, D, np.asarray(entity), np.asarray(edge_index),
                np.asarray(edge_type), np.asarray(edge_norm),
                np.asarray(att1), np.asarray(att2))
    NPC = prm["NPC"]

    entity = np.asarray(entity, dtype=np.float32)

    iota_arr = np.tile(np.arange(BLK, dtype=np.float32), (TPE, CHK)).astype(BF)
    iota2_arr = np.tile(np.arange(BLK2, dtype=np.float32), (TPE, 8)).astype(BF)
    iotas_arr = np.tile(np.arange(2 * GRP, dtype=np.float32), (TPE, 1)).astype(BF)
    ident_arr = np.eye(TPE, dtype=np.float32).astype(BF)
    b1 = np.ascontiguousarray(
        np.asarray(basis1, np.float32).transpose(1, 0, 2).reshape(D, 4 * D)).astype(BF)
    b2 = np.ascontiguousarray(
        np.asarray(basis2, np.float32).transpose(1, 0, 2).reshape(D, 4 * D)).astype(BF)

    nc = _build(prm, D)

    in_maps = []
    for m in range(M):
        lo, hi = m * NPC, min((m + 1) * NPC, N)
        xs = np.zeros((NPC, D), dtype=np.float32)
        if hi > lo:
            xs[0:hi - lo] = entity[lo:hi]
        xslp_arr = np.ascontiguousarray(
            xs.reshape(prm["NGRP"], GRP, D).transpose(2, 0, 1)
            .reshape(D, prm["NGRP"] * GRP)).astype(BF)
        in_maps.append({
            "xslp": xslp_arr,
            "xpg": prm["xpg"][m],
            "idxp": prm["idxp"][m],
            "metac1": prm["metac1"][m],
            "metac2": prm["metac2"][m],
            "iota": iota_arr,
            "iota2": iota2_arr,
            "iotas": iotas_arr,
            "ident": ident_arr,
            "bas1": b1,
            "bas2": b2,
            "rt1": np.asarray(root1, np.float32).astype(BF),
            "rt2": np.asarray(root2, np.float32).astype(BF),
            "bias1": np.tile(np.asarray(bias1, np.float32), (GRP, 1)),
            "bias2": np.tile(np.asarray(bias2, np.float32), (GRP, 1)),
        })
    kwargs = {}
    if TRACE:
        kwargs = dict(trace=True, tmpdir=TRACE_DIR)
    res = run_bass_kernel_spmd(nc, in_maps, core_ids=list(range(M)), **kwargs)
    global LAST
    LAST = res
    out = np.concatenate([res.results[m]["outp"] for m in range(M)], axis=0)
    return np.ascontiguousarray(out[:N])


LAST = None
TRACE = False
TRACE_DIR = None



# revision 17
# speedup vs baseline: 1.0823x; 1.0123x over previous
"""RGCN (2-layer, basis-decomposition) Trainium2 kernel, v4.

Strategy (8 NeuronCores, SPMD), building on v2/v3:
  - Edges sorted by destination; dst nodes partitioned into 8 contiguous
    ranges (one per core). Messages + segment-sum fused into per-tile PE
    matmuls against a DVE-built "weighted one-hot" (woh) matrix.
  - v3: layer-2 gather descriptors are PREPARED (prepare_only) on 4 SWDGE
    queues during layer 1, and each page's DMA is fired by a cheap
    trigger_dma once the AllGathered h table is ready; gather desc-gen is
    off the critical path.
  - v4: both layers use per-group cap+spill tiling (L1: 4 blocks of 32 x
    2 tiles + 1 spill tile per group; L2: 8 blocks of 16 x 1 tile + 1
    spill tile per group). Narrow blocks halve the DVE one-hot/multiply
    width per edge; spill tiles use a permuted group-wide one-hot that
    matches the (bi, basis, j) psum column layout.
"""

import math

import numpy as np
import ml_dtypes

import concourse.bacc as bacc
import concourse.bass as bass
import concourse.mybir as mybir
import concourse.tile as tile
from concourse.bass_utils import run_bass_kernel_spmd

F32 = mybir.dt.float32
BF16 = mybir.dt.bfloat16
I16 = mybir.dt.int16
AF = mybir.ActivationFunctionType
ALU = mybir.AluOpType
BF = ml_dtypes.bfloat16

M = 8            # cores
GRP = 128        # nodes per output group
TPE = 128        # edges per tile
GP = 36          # tiles per page (4 groups of 9)
CHK = 8          # base tiles per group (= one woh build chunk)
DW = 128         # padded table row width (256 B in bf16)
BLK1 = 32        # layer-1 block size
CAP1 = 2         # layer-1 base tiles per block
BLK2 = 16        # layer-2 block size
CAP2 = 1         # layer-2 base tiles per block
NCHUNK = 4       # AllGather chunks
GB_BUFS = 4      # resident layer-2 gather pages (WAR window)
NSUB = 4         # sub-preps per gather page
PREP_PAGES = 2   # pages desc-prepared during layer 1 (ring capacity bound)

TAIL_DELAY = 10   # chunks between a group's last matmul and its tail
TAIL_DELAY2 = 4   # layer-2 variant


def _expand(ap, free_dims, col_offset=0):
    """AP with the partition dim kept and explicit [step, count] free dims."""
    base = ap.ap
    return bass.AP(
        ap.tensor,
        ap.offset + col_offset,
        [list(base[0])] + [list(d) for d in free_dims],
    )


def _page_subranges(nt):
    base, rem = divmod(nt, NSUB)
    out, t = [], 0
    for i in range(NSUB):
        s = base + (1 if i < rem else 0)
        out.append((t, t + s))
        t += s
    return [r for r in out if r[1] > r[0]]


def _prep(N, D, entity, edge_index, edge_type, edge_norm, att1, att2):
    """Host-side graph preprocessing. Returns per-core arrays + structure."""
    NPC = int(math.ceil(N / (M * GRP))) * GRP      # nodes per core (6272)
    SEG = NPC + GRP                                # table segment per core
    NTAB = SEG * M                                 # 51200
    NGRP = NPC // GRP                              # 49

    # AllGather chunking of the 49 groups
    base_cg = NGRP // NCHUNK
    cg = [base_cg] * NCHUNK
    cg[0] += NGRP - base_cg * NCHUNK               # [13,12,12,12]
    chunk_of_group = np.repeat(np.arange(NCHUNK), cg)
    chunk_gstart = np.concatenate([[0], np.cumsum(cg)[:-1]])
    pr_c = [x * (GRP // 2) for x in cg]
    pr_c[0] += GRP // 2                            # zero region pair rows
    hfull_base = np.concatenate([[0], np.cumsum([p * M for p in pr_c])[:-1]])
    zero_row = int(hfull_base[0] + pr_c[0] - GRP // 2)

    src = np.asarray(edge_index[0], dtype=np.int64)
    dst = np.asarray(edge_index[1], dtype=np.int64)
    et = np.asarray(edge_type[:, 0], dtype=np.int64)
    norm = np.asarray(edge_norm, dtype=np.float32)

    order = np.argsort(dst, kind="stable")
    src_s, dst_s, et_s, norm_s = src[order], dst[order], et[order], norm[order]

    deg = np.bincount(dst, minlength=NPC * M).astype(np.float32)
    inv_deg = (1.0 / np.maximum(deg, 1.0)).astype(np.float32)
    core_of = dst_s // NPC

    def capspill(BLKx, CAP):
        """Per-group cap+spill tiling: per group bpg*CAP base tiles (one
        per (block, w)) + 1 spill tile; groups of 9 tiles, pages of GP."""
        bpg = GRP // BLKx
        nblk = NPC // BLKx
        tiles, chunks = [], []
        for g in range(NGRP):
            chunks.append(("s", 1, len(tiles)))
            tiles.append(("s", g, 0, 0))
            chunks.append(("b", bpg * CAP, len(tiles)))
            for bi in range(bpg):
                for w in range(CAP):
                    tiles.append(("b", g, bi, w))
        T = len(tiles)
        npage = -(-T // GP)
        Tpad = npage * GP
        page_tiles = [min(GP, T - p * GP) for p in range(npage)]
        base_tile_id = np.zeros((nblk, CAP), np.int64)
        spill_tile_id = np.zeros(NGRP, np.int64)
        for t, (kind, g, bi, w) in enumerate(tiles):
            if kind == "s":
                spill_tile_id[g] = t
            else:
                base_tile_id[g * bpg + bi, w] = t
        percore = []
        for m in range(M):
            eidx = np.nonzero(core_of == m)[0]
            dl = dst_s[eidx] - m * NPC
            blk = dl // BLKx
            es = np.zeros(nblk + 1, np.int64)
            es[1:] = np.cumsum(np.bincount(blk, minlength=nblk))
            w2 = np.arange(len(eidx)) - es[blk]
            is_base = w2 < CAP * TPE
            sl = np.empty(len(eidx), np.int64)
            sl[is_base] = (base_tile_id[blk[is_base], w2[is_base] // TPE]
                           * TPE + w2[is_base] % TPE)
            sp = ~is_base
            sp_cnt = np.maximum(0, np.diff(es) - CAP * TPE)
            g_of = np.arange(nblk) // bpg
            tot = np.cumsum(sp_cnt) - sp_cnt
            first = np.searchsorted(g_of, np.arange(NGRP))
            sp_base = tot - tot[first][g_of]
            sw = w2[sp] - CAP * TPE + sp_base[blk[sp]]
            assert sw.max(initial=0) < TPE, ("spill overflow",
                                             int(sw.max(initial=0)))
            sl[sp] = spill_tile_id[g_of[blk[sp]]] * TPE + sw
            do = np.where(is_base, dl % BLKx, dl % GRP).astype(np.float32)
            percore.append((eidx, sl, do))
        return dict(chunks=chunks, tiles=tiles, T=T, Tpad=Tpad,
                    NPAGE=npage, page_tiles=page_tiles, percore=percore,
                    bpg=bpg)

    L1 = capspill(BLK1, CAP1)
    L2 = capspill(BLK2, CAP2)

    nd = norm_s * inv_deg[dst_s]
    c1 = (att1[et_s] * nd[:, None]).astype(np.float32)
    c2 = (att2[et_s] * nd[:, None]).astype(np.float32)

    # layer-2 gather: chunk-major hfull pair-row index for each source node
    k_s = src_s // NPC
    n_s = src_s % NPC
    g_s = n_s // GRP
    c_s = chunk_of_group[g_s]
    lpr = (g_s - chunk_gstart[c_s]) * (GRP // 2) + (n_s % GRP) // 2
    row_s = hfull_base[c_s] + k_s * np.asarray(pr_c)[c_s] + lpr
    par_s = (n_s % 2).astype(np.float32)

    def pack_idx(lin, npage):
        """[npage*GP*TPE] indices -> [npage, 128, GP*8] int16 layout."""
        out = np.empty((npage, 16, GP * 8), np.int16)
        lp = lin.reshape(npage, GP * TPE)
        out[:, :, :] = lp.reshape(npage, GP * 8, 16).transpose(0, 2, 1)
        return np.ascontiguousarray(np.tile(out, (1, 8, 1)))

    def scat(vals, sl, Tpad, fill=0.0, width=None):
        if width is None:
            out = np.full(Tpad * TPE, fill, dtype=np.float32)
        else:
            out = np.full((Tpad * TPE, width), fill, dtype=np.float32)
        out[sl] = vals
        return out

    def tilemaj(a, npage, w):
        """[Tpad*TPE(,w)] slot-major -> [npage, TPE, GP*w] tile-major."""
        if a.ndim == 1:
            a = a[:, None]
        return np.ascontiguousarray(
            a.reshape(npage, GP, TPE, a.shape[1]).transpose(0, 2, 1, 3)
            .reshape(npage, TPE, GP * a.shape[1]).astype(BF))

    ent = np.asarray(entity, dtype=np.float32)

    metac1s, metac2s, idx_pages, xpgs = [], [], [], []
    for m in range(M):
        eidx, sl1, do1 = L1["percore"][m]
        NP1, TP1 = L1["NPAGE"], L1["Tpad"]
        xsrcm = np.zeros((TP1 * TPE, D), dtype=BF)
        xsrcm[sl1] = ent[src_s[eidx]].astype(BF)
        xpgs.append(tilemaj(xsrcm.astype(np.float32), NP1, D))
        metac1 = np.concatenate([
            tilemaj(scat(do1, sl1, TP1, 320.0), NP1, 1),
            tilemaj(scat(c1[eidx], sl1, TP1, width=4), NP1, 4)], axis=2)
        metac1s.append(np.ascontiguousarray(metac1))

        eidx2, sl2, do2 = L2["percore"][m]
        NP2, TP2 = L2["NPAGE"], L2["Tpad"]
        rows = np.full(TP2 * TPE, zero_row, dtype=np.int64)
        rows[sl2] = row_s[eidx2]
        idx_pages.append(pack_idx(rows.astype(np.int16), NP2))
        par = scat(par_s[eidx2], sl2, TP2)
        ca = scat(c2[eidx2], sl2, TP2, width=4)
        metac2 = np.concatenate([
            tilemaj(scat(do2, sl2, TP2, 320.0), NP2, 1),
            tilemaj(ca * (1.0 - par)[:, None], NP2, 4),
            tilemaj(ca * par[:, None], NP2, 4)], axis=2)
        metac2s.append(np.ascontiguousarray(metac2))

    return dict(NPC=NPC, SEG=SEG, NTAB=NTAB, NGRP=NGRP, L1=L1, L2=L2,
                cg=cg, chunk_of_group=chunk_of_group,
                chunk_gstart=chunk_gstart, pr_c=pr_c, hfull_base=hfull_base,
                idxp=idx_pages, metac1=metac1s, metac2=metac2s, xpg=xpgs)


def _chunks_by_page(L):
    out = [[] for _ in range(L["NPAGE"])]
    for kind, chks, toff in L["chunks"]:
        out[toff // GP].append((kind, chks, toff))
    return out


def _make_prepper(nc, prm, gath2, table_ap, idxt_all, gsems, psems):
    """Prepare-only gather desc-gen for the first PREP_PAGES layer-2 pages.

    Each page's NSUB sub-preps are spread round-robin across the 4 SWDGE
    queues (keeps parked-descriptor count per ring low and lets the 4 Q7
    core pairs generate concurrently). trigger(q) fires page q's subs
    (count=1 on each queue, FIFO order); gwait(q) gates the tensor engine
    on the page's gather DMAs. Later pages use plain direct gathers."""
    page_tiles = prm["L2"]["page_tiles"]
    gbufs = {}
    nsub_q = [0, 0, 0, 0]       # subs emitted per queue
    ntrig_q = [0, 0, 0, 0]      # subs triggered per queue

    def emit_page_preps(q):
        if q not in gbufs:
            gbufs[q] = gath2.tile([TPE, GP, 2 * DW], BF16, tag="gbuf2",
                                  bufs=GB_BUFS, name="gbuf2")
        for si, (t0, t1) in enumerate(_page_subranges(page_tiles[q])):
            qq = si % 4
            nidx = (t1 - t0) * TPE
            nc.gpsimd.dma_gather(
                out_ap=gbufs[q][:, t0:t1, :], in_ap=table_ap,
                idxs_ap=idxt_all[:, q, t0 * 8:t1 * 8],
                num_idxs=nidx, num_idxs_reg=nidx,
                elem_size=2 * DW, single_packet=False, queue_num=qq,
                prepare_only=True, sem=gsems[qq]).then_inc(psems[qq], 1)
            nsub_q[qq] += 1

    def emit_direct(q):
        qq = q % 4
        if q not in gbufs:
            gbufs[q] = gath2.tile([TPE, GP, 2 * DW], BF16, tag="gbuf2",
                                  bufs=GB_BUFS, name="gbuf2")
        nt = page_tiles[q]
        nidx = nt * TPE
        nc.gpsimd.dma_gather(
            out_ap=gbufs[q][:, 0:nt, :], in_ap=table_ap,
            idxs_ap=idxt_all[:, q, 0:nt * 8],
            num_idxs=nidx, num_idxs_reg=nidx,
            elem_size=2 * DW, single_packet=False, queue_num=qq)

    def trigger(q):
        nsubs = len(_page_subranges(page_tiles[q]))
        for si in range(nsubs):
            qq = si % 4
            ntrig_q[qq] += 1
            nc.gpsimd.wait_ge(psems[qq], ntrig_q[qq])
            nc.gpsimd.trigger_dma(count=1, queue_num=qq)

    def gwait(q):
        for qq in range(4):
            nc.tensor.wait_ge(gsems[qq], 16 * (q + 1))

    return emit_page_preps, trigger, gbufs, emit_direct, gwait


def _layer1(tc, nc, pools, prm, D, xpg, metac1_all, iota1_t, iota128_t,
            ident_t, bas_t, rt_t, bias_t, xslb, h4buf, hsl_cs, group_done,
            prep_hook):
    """Dense layer 1: pre-gathered x rows; per-group spill + base tiles."""
    gath, wohp, hp, xtp, sbigp, psp = pools
    L = prm["L1"]
    NPAGE, tiles, bpg = L["NPAGE"], L["tiles"], L["bpg"]
    cbp = _chunks_by_page(L)

    pending = []

    def flush(now):
        while pending and (now is None or pending[0][0] + TAIL_DELAY <= now):
            _, g, sbig = pending.pop(0)
            _tail1(tc, nc, pools, prm, D, g, sbig, xslb, h4buf,
                   ident_t, bas_t, rt_t, bias_t, hsl_cs)
            group_done(g)
            prep_hook()

    psums = {}
    cglob = 0
    for q in range(NPAGE):
        gbuf = gath.tile([TPE, GP * D], BF16, tag="gbuf1", bufs=3)
        nc.sync.dma_start(out=gbuf[:], in_=xpg[q])
        metat = metac1_all[:, q, :]
        for kind, chks, toff in cbp[q]:
            flush(cglob)
            cglob += 1
            loff = toff - q * GP
            if kind == "s":
                g = tiles[toff][1]
                ohs = wohp.tile([TPE, GRP], BF16, tag="ohs1", bufs=3)
                nc.vector.tensor_tensor(
                    out=ohs[:], in0=iota128_t,
                    in1=_expand(metat, [[0, GRP]], col_offset=loff),
                    op=ALU.is_equal)
                wohs = wohp.tile([TPE, 4 * GRP], BF16, tag="wohs1", bufs=3)
                for b in range(4):
                    nc.vector.tensor_tensor(
                        out=_expand(wohs[:], [[4 * BLK1, bpg], [1, BLK1]],
                                    col_offset=b * BLK1),
                        in0=_expand(ohs[:], [[BLK1, bpg], [1, BLK1]]),
                        in1=_expand(metat, [[0, bpg], [0, BLK1]],
                                    col_offset=GP + loff * 4 + b),
                        op=ALU.mult)
                psum_blk = psp.tile([D, 4 * GRP], F32, tag="blk", bufs=3,
                                    name="psum_blk")
                psums[g] = psum_blk
                nc.tensor.matmul(out=psum_blk[:],
                                 lhsT=gbuf[:, loff * D:(loff + 1) * D],
                                 rhs=wohs[:], start=True, stop=False)
            else:
                oh8 = wohp.tile([TPE, CHK * BLK1], BF16, tag="oh1", bufs=4)
                nc.vector.tensor_tensor(
                    out=_expand(oh8[:], [[BLK1, CHK], [1, BLK1]]),
                    in0=iota1_t,
                    in1=_expand(metat, [[1, CHK], [0, BLK1]],
                                col_offset=loff),
                    op=ALU.is_equal)
                woh = wohp.tile([TPE, CHK * 4 * BLK1], BF16, tag="woh1",
                                bufs=4)
                for b in range(4):
                    nc.vector.tensor_tensor(
                        out=_expand(woh[:], [[4 * BLK1, CHK], [1, BLK1]],
                                    col_offset=b * BLK1),
                        in0=_expand(oh8[:], [[BLK1, CHK], [1, BLK1]]),
                        in1=_expand(metat, [[4, CHK], [0, BLK1]],
                                    col_offset=GP + loff * 4 + b),
                        op=ALU.mult)
                for u in range(CHK):
                    _, g, bi, w = tiles[toff + u]
                    last = (bi == bpg - 1 and w == CAP1 - 1)
                    nc.tensor.matmul(
                        out=psums[g][:, bi * 4 * BLK1:(bi + 1) * 4 * BLK1],
                        lhsT=gbuf[:, (loff + u) * D:(loff + u + 1) * D],
                        rhs=woh[:, u * 4 * BLK1:(u + 1) * 4 * BLK1],
                        start=False, stop=last)
                    if last:
                        psum_blk = psums.pop(g)
                        sbig = sbigp.tile([D, 4 * GRP], BF16, tag="sbig",
                                          bufs=5)
                        for b in range(4):
                            nc.scalar.copy(
                                out=_expand(sbig[:], [[BLK1, bpg], [1, BLK1]],
                                            col_offset=b * GRP),
                                in_=_expand(psum_blk[:],
                                            [[4 * BLK1, bpg], [1, BLK1]],
                                            col_offset=b * BLK1))
                        pending.append((cglob, g, sbig))
    flush(None)


def _tail1(tc, nc, pools, prm, D, g, sbig, xslb, h4buf, ident_t,
           bas_t, rt_t, bias_t, hsl_cs):
    """Layer-1 group tail: combine bases, mean, root, bias, relu -> hsl."""
    gath, wohp, hp, xtp, sbigp, psp = pools
    pagg = psp.tile([GRP, D], F32, tag="agg", bufs=2)
    for b in range(4):
        nc.tensor.matmul(
            out=pagg[:],
            lhsT=sbig[:, b * GRP:(b + 1) * GRP],
            rhs=bas_t[:, b * D:(b + 1) * D],
            start=(b == 0),
            stop=False,
        )
    nc.tensor.matmul(out=pagg[:], lhsT=xslb[:, g, :], rhs=rt_t,
                     start=False, stop=True)

    h3 = hp.tile([GRP, D], F32, tag="h3")
    nc.vector.tensor_tensor(out=h3[:], in0=pagg[:], in1=bias_t, op=ALU.add)

    nc.scalar.activation(out=h4buf[:, g, 0:D], in_=h3[:], func=AF.Relu)
    c = int(prm["chunk_of_group"][g])
    gl = g - int(prm["chunk_gstart"][c])
    base = hsl_cs[c][:]
    dst = bass.AP(base.tensor, base.offset + gl * (GRP // 2) * (2 * DW),
                  [[2 * DW, GRP // 2], [DW, 2], [1, DW]])
    nc.sync.dma_start(out=dst, in_=h4buf[:, g, :])


def _layer2(tc, nc, pools, prm, D, metat_all,
            iota2_t, iota128_t, ident_t, bas_t, rt_t, bias_t, h4buf,
            out_ap, trigger, gbufs, gwait, emit_direct):
    """Gather-based layer 2: per-group spill + 8 base tiles of 16."""
    gath, wohp, hp, xtp, sbigp, psp = pools
    L = prm["L2"]
    NPAGE, tiles, bpg = L["NPAGE"], L["tiles"], L["bpg"]
    cbp = _chunks_by_page(L)

    pending = []

    def flush(now):
        while pending and (now is None or pending[0][0] + TAIL_DELAY2 <= now):
            _, g, sbig = pending.pop(0)
            _tail2(tc, nc, pools, prm, D, g, sbig, h4buf, ident_t,
                   bas_t, rt_t, bias_t, out_ap)

    for q in range(min(GB_BUFS, NPAGE)):
        emit_direct(q)

    psums = {}
    cglob = 0
    for q in range(NPAGE):
        if q > 0 and q - 1 + GB_BUFS < NPAGE:
            emit_direct(q - 1 + GB_BUFS)
        metat = metat_all[:, q, :]
        gbuf = gbufs[q]
        for kind, chks, toff in cbp[q]:
            flush(cglob)
            cglob += 1
            loff = toff - q * GP
            if kind == "s":
                g = tiles[toff][1]
                ohs = wohp.tile([TPE, GRP], BF16, tag="ohs2", bufs=3)
                nc.vector.tensor_tensor(
                    out=ohs[:], in0=iota128_t,
                    in1=_expand(metat, [[0, GRP]], col_offset=loff),
                    op=ALU.is_equal)
                wohsE = wohp.tile([TPE, 4 * GRP], BF16, tag="wohsE", bufs=3)
                wohsO = wohp.tile([TPE, 4 * GRP], BF16, tag="wohsO", bufs=3)
                for woh, cbase in ((wohsE, GP), (wohsO, 5 * GP)):
                    for b in range(4):
                        nc.vector.tensor_tensor(
                            out=_expand(woh[:], [[4 * BLK2, bpg], [1, BLK2]],
                                        col_offset=b * BLK2),
                            in0=_expand(ohs[:], [[BLK2, bpg], [1, BLK2]]),
                            in1=_expand(metat, [[0, bpg], [0, BLK2]],
                                        col_offset=cbase + loff * 4 + b),
                            op=ALU.mult)
                psum_blk = psp.tile([D, 4 * GRP], F32, tag="blk", bufs=3,
                                    name="psum_blk")
                psums[g] = psum_blk
                nc.tensor.matmul(out=psum_blk[:], lhsT=gbuf[:, loff, 0:D],
                                 rhs=wohsE[:], start=True, stop=False)
                nc.tensor.matmul(out=psum_blk[:],
                                 lhsT=gbuf[:, loff, DW:DW + D],
                                 rhs=wohsO[:], start=False, stop=False)
            else:
                oh8 = wohp.tile([TPE, CHK * BLK2], BF16, tag="oh2", bufs=5)
                nc.vector.tensor_tensor(
                    out=_expand(oh8[:], [[BLK2, CHK], [1, BLK2]]),
                    in0=iota2_t,
                    in1=_expand(metat, [[1, CHK], [0, BLK2]],
                                col_offset=loff),
                    op=ALU.is_equal)
                wohE = wohp.tile([TPE, CHK * 4 * BLK2], BF16, tag="wohE",
                                 bufs=4)
                wohO = wohp.tile([TPE, CHK * 4 * BLK2], BF16, tag="wohO",
                                 bufs=4)
                for woh, cbase in ((wohE, GP), (wohO, 5 * GP)):
                    for b in range(4):
                        nc.vector.tensor_tensor(
                            out=_expand(woh[:], [[4 * BLK2, CHK], [1, BLK2]],
                                        col_offset=b * BLK2),
                            in0=_expand(oh8[:], [[BLK2, CHK], [1, BLK2]]),
                            in1=_expand(metat, [[4, CHK], [0, BLK2]],
                                        col_offset=cbase + loff * 4 + b),
                            op=ALU.mult)
                for u in range(CHK):
                    _, g, bi, w = tiles[toff + u]
                    stop = (bi == bpg - 1)
                    out_sl = psums[g][:, bi * 4 * BLK2:(bi + 1) * 4 * BLK2]
                    nc.tensor.matmul(out=out_sl,
                                     lhsT=gbuf[:, loff + u, 0:D],
                                     rhs=wohE[:, u * 4 * BLK2:(u + 1) * 4 * BLK2],
                                     start=False, stop=False)
                    nc.tensor.matmul(out=out_sl,
                                     lhsT=gbuf[:, loff + u, DW:DW + D],
                                     rhs=wohO[:, u * 4 * BLK2:(u + 1) * 4 * BLK2],
                                     start=False, stop=stop)
                    if stop:
                        psum_blk = psums.pop(g)
                        sbig = sbigp.tile([D, 4 * GRP], BF16, tag="sbig",
                                          bufs=5)
                        for b in range(4):
                            nc.scalar.copy(
                                out=_expand(sbig[:], [[BLK2, bpg], [1, BLK2]],
                                            col_offset=b * GRP),
                                in_=_expand(psum_blk[:],
                                            [[4 * BLK2, bpg], [1, BLK2]],
                                            col_offset=b * BLK2))
                        pending.append((cglob, g, sbig))
    flush(None)


def _tail2(tc, nc, pools, prm, D, g, sbig, h4buf, ident_t,
           bas_t, rt_t, bias_t, out_ap):
    gath, wohp, hp, xtp, sbigp, psp = pools
    ptr = psp.tile([D, GRP], BF16, tag="tr", bufs=2)
    nc.tensor.transpose(out=ptr[:], in_=h4buf[:, g, 0:D], identity=ident_t)
    xt = xtp.tile([D, GRP], BF16, tag="xt")
    nc.scalar.copy(out=xt[:], in_=ptr[:])

    pagg = psp.tile([GRP, D], F32, tag="agg", bufs=2)
    for b in range(4):
        nc.tensor.matmul(
            out=pagg[:],
            lhsT=sbig[:, b * GRP:(b + 1) * GRP],
            rhs=bas_t[:, b * D:(b + 1) * D],
            start=(b == 0),
            stop=False,
        )
    nc.tensor.matmul(out=pagg[:], lhsT=xt[:], rhs=rt_t, start=False, stop=True)

    h3 = hp.tile([GRP, D], F32, tag="h3")
    nc.vector.tensor_tensor(out=h3[:], in0=pagg[:], in1=bias_t, op=ALU.add)
    nc.scalar.dma_start(out=out_ap[g * GRP:(g + 1) * GRP, :], in_=h3[:])


def _build(prm, D):
    NPC, NTAB = prm["NPC"], prm["NTAB"]
    NGRP = prm["NGRP"]
    NPAGE1, NPAGE2 = prm["L1"]["NPAGE"], prm["L2"]["NPAGE"]
    pr_c, cg = prm["pr_c"], prm["cg"]
    nc = bacc.Bacc(num_swdge_queues=4)

    xslp = nc.dram_tensor("xslp", [D, NGRP * GRP], BF16, kind="ExternalInput")
    xpg = nc.dram_tensor("xpg", [NPAGE1, TPE, GP * D], BF16,
                         kind="ExternalInput")
    idxp = nc.dram_tensor("idxp", [NPAGE2, TPE, 8 * GP], I16,
                          kind="ExternalInput")
    metac1 = nc.dram_tensor("metac1", [NPAGE1, TPE, 5 * GP], BF16,
                            kind="ExternalInput")
    metac2 = nc.dram_tensor("metac2", [NPAGE2, TPE, 9 * GP], BF16,
                            kind="ExternalInput")
    iota1 = nc.dram_tensor("iota1", [TPE, CHK * BLK1], BF16,
                           kind="ExternalInput")
    iota2 = nc.dram_tensor("iota2", [TPE, CHK * BLK2], BF16,
                           kind="ExternalInput")
    iota128 = nc.dram_tensor("iota128", [TPE, GRP], BF16,
                             kind="ExternalInput")
    ident = nc.dram_tensor("ident", [TPE, TPE], BF16, kind="ExternalInput")
    bas1 = nc.dram_tensor("bas1", [D, 4 * D], BF16, kind="ExternalInput")
    bas2 = nc.dram_tensor("bas2", [D, 4 * D], BF16, kind="ExternalInput")
    rt1 = nc.dram_tensor("rt1", [D, D], BF16, kind="ExternalInput")
    rt2 = nc.dram_tensor("rt2", [D, D], BF16, kind="ExternalInput")
    bias1 = nc.dram_tensor("bias1", [GRP, D], F32, kind="ExternalInput")
    bias2 = nc.dram_tensor("bias2", [GRP, D], F32, kind="ExternalInput")
    outp = nc.dram_tensor("outp", [NPC, D], F32, kind="ExternalOutput")

    with tile.TileContext(nc) as tc:
        with (
            tc.tile_pool(name="const", bufs=1) as cst,
            tc.tile_pool(name="gath", bufs=3) as gath,
            tc.tile_pool(name="woh", bufs=3) as wohp,
            tc.tile_pool(name="hp", bufs=3) as hp,
            tc.tile_pool(name="xtp", bufs=3) as xtp,
            tc.tile_pool(name="sbig", bufs=2) as sbigp,
            tc.tile_pool(name="ps", bufs=1, space="PSUM") as psp,
            tc.tile_pool(name="dram", bufs=1, space="DRAM") as dramp,
        ):
            pools = (gath, wohp, hp, xtp, sbigp, psp)

            iota1_t = cst.tile([TPE, CHK * BLK1], BF16)
            nc.sync.dma_start(out=iota1_t[:], in_=iota1[:])
            iota2_t = cst.tile([TPE, CHK * BLK2], BF16)
            nc.sync.dma_start(out=iota2_t[:], in_=iota2[:])
            iota128_t = cst.tile([TPE, GRP], BF16)
            nc.sync.dma_start(out=iota128_t[:], in_=iota128[:])
            xslb = cst.tile([D, NGRP, GRP], BF16)
            nc.sync.dma_start(out=xslb[:], in_=xslp[:])
            h4buf = cst.tile([GRP, NGRP, DW], BF16)
            nc.vector.memset(h4buf[:], 0.0)
            ident_t = cst.tile([TPE, TPE], BF16)
            nc.sync.dma_start(out=ident_t[:], in_=ident[:])
            bas1_t = cst.tile([D, 4 * D], BF16)
            nc.sync.dma_start(out=bas1_t[:], in_=bas1[:])
            bas2_t = cst.tile([D, 4 * D], BF16)
            nc.sync.dma_start(out=bas2_t[:], in_=bas2[:])
            rt1_t = cst.tile([D, D], BF16)
            nc.sync.dma_start(out=rt1_t[:], in_=rt1[:])
            rt2_t = cst.tile([D, D], BF16)
            nc.sync.dma_start(out=rt2_t[:], in_=rt2[:])
            bias1_t = cst.tile([GRP, D], F32)
            nc.sync.dma_start(out=bias1_t[:], in_=bias1[:])
            bias2_t = cst.tile([GRP, D], F32)
            nc.sync.dma_start(out=bias2_t[:], in_=bias2[:])
            zed_t = cst.tile([GRP, 2 * DW], BF16)
            nc.gpsimd.memset(zed_t[:], 0.0)

            # preloaded layer-2 gather indices + both layers' meta pages
            idxt_all = cst.tile([TPE, NPAGE2, 8 * GP], I16)
            ib = idxp[:]
            nc.sync.dma_start(
                out=idxt_all[:],
                in_=bass.AP(ib.tensor, ib.offset,
                            [[8 * GP, TPE], [TPE * 8 * GP, NPAGE2],
                             [1, 8 * GP]]))
            metat2_all = cst.tile([TPE, NPAGE2, 9 * GP], BF16)
            mb2 = metac2[:]
            nc.sync.dma_start(
                out=metat2_all[:],
                in_=bass.AP(mb2.tensor, mb2.offset,
                            [[9 * GP, TPE], [TPE * 9 * GP, NPAGE2],
                             [1, 9 * GP]]))
            metac1_all = cst.tile([TPE, NPAGE1, 5 * GP], BF16)
            mb1 = metac1[:]
            nc.sync.dma_start(
                out=metac1_all[:],
                in_=bass.AP(mb1.tensor, mb1.offset,
                            [[5 * GP, TPE], [TPE * 5 * GP, NPAGE1],
                             [1, 5 * GP]]))

            hsl_cs = [dramp.tile([pr_c[c], 2 * DW], BF16, name=f"hsl_c{c}")
                      for c in range(NCHUNK)]
            hfull = dramp.tile([NTAB // 2, 2 * DW], BF16)

            # zero rows at the tail of chunk 0 (dummy gather targets)
            nc.sync.dma_start(
                out=hsl_cs[0][pr_c[0] - GRP // 2:pr_c[0], :],
                in_=zed_t[0:GRP // 2, :])

            gsems = [nc.alloc_semaphore(f"gsem{q}") for q in range(4)]
            psems = [nc.alloc_semaphore(f"psem{q}") for q in range(4)]
            for s in gsems + psems:
                nc.gpsimd.sem_clear(s)
            hchk = cst.tile([1, 2 * DW], BF16)
            hchk2 = cst.tile([1, 2 * DW], BF16)

            fired = [False] * NCHUNK
            last_group_of_chunk = np.cumsum(cg) - 1

            def group_done(g):
                for c in range(NCHUNK):
                    if g == last_group_of_chunk[c] and not fired[c]:
                        fired[c] = True
                        base = int(prm["hfull_base"][c] // 1)
                        rows = pr_c[c] * M
                        nc.gpsimd.collective_compute(
                            "AllGather",
                            ALU.bypass,
                            replica_groups=[list(range(M))],
                            ins=[hsl_cs[c][:]],
                            outs=[hfull[base:base + rows, :]],
                        )

            with tc.tile_pool(name="gath2", bufs=GB_BUFS) as gath2:
                (emit_page_preps, trigger, gbufs, emit_direct,
                 gwait) = _make_prepper(
                    nc, prm, gath2, hfull[:, :], idxt_all, gsems, psems)

                prep_q = []

                def prep_hook():
                    pass

                _layer1(tc, nc, pools, prm, D, xpg, metac1_all,
                        iota1_t[:], iota128_t[:], ident_t[:], bas1_t,
                        rt1_t[:], bias1_t[:], xslb, h4buf, hsl_cs,
                        group_done, prep_hook)

                _layer2(tc, nc, pools, prm, D, metat2_all,
                        iota2_t[:], iota128_t[:], ident_t[:], bas2_t,
                        rt2_t[:], bias2_t[:], h4buf, outp,
                        trigger, gbufs, gwait, emit_direct)
    nc.compile()
    return nc


def kernel(entity, edge_index, edge_attr, edge_type, edge_norm,
           basis1, att1, root1, bias1, basis2, att2, root2, bias2):
    N, D = entity.shape
    prm = _prep(N, D, np.asarray(entity), np.asarray(edge_index),
                np.asarray(edge_type), np.asarray(edge_norm),
                np.asarray(att1), np.asarray(att2))
    NPC = prm["NPC"]

    entity = np.asarray(entity, dtype=np.float32)

    iota1_arr = np.tile(np.arange(BLK1, dtype=np.float32), (TPE, CHK)).astype(BF)
    iota2_arr = np.tile(np.arange(BLK2, dtype=np.float32), (TPE, CHK)).astype(BF)
    iota128_arr = np.tile(np.arange(GRP, dtype=np.float32), (TPE, 1)).astype(BF)
    ident_arr = np.eye(TPE, dtype=np.float32).astype(BF)
    b1 = np.ascontiguousarray(
        np.asarray(basis1, np.float32).transpose(1, 0, 2).reshape(D, 4 * D)).astype(BF)
    b2 = np.ascontiguousarray(
        np.asarray(basis2, np.float32).transpose(1, 0, 2).reshape(D, 4 * D)).astype(BF)

    nc = _build(prm, D)

    in_maps = []
    for m in range(M):
        lo, hi = m * NPC, min((m + 1) * NPC, N)
        xs = np.zeros((NPC, D), dtype=np.float32)
        if hi > lo:
            xs[0:hi - lo] = entity[lo:hi]
        xslp_arr = np.ascontiguousarray(
            xs.reshape(prm["NGRP"], GRP, D).transpose(2, 0, 1)
            .reshape(D, prm["NGRP"] * GRP)).astype(BF)
        in_maps.append({
            "xslp": xslp_arr,
            "xpg": prm["xpg"][m],
            "idxp": prm["idxp"][m],
            "metac1": prm["metac1"][m],
            "metac2": prm["metac2"][m],
            "iota1": iota1_arr,
            "iota2": iota2_arr,
            "iota128": iota128_arr,
            "ident": ident_arr,
            "bas1": b1,
            "bas2": b2,
            "rt1": np.asarray(root1, np.float32).astype(BF),
            "rt2": np.asarray(root2, np.float32).astype(BF),
            "bias1": np.tile(np.asarray(bias1, np.float32), (GRP, 1)),
            "bias2": np.tile(np.asarray(bias2, np.float32), (GRP, 1)),
        })
    kwargs = {}
    if TRACE:
        kwargs = dict(trace=True, tmpdir=TRACE_DIR)
    res = run_bass_kernel_spmd(nc, in_maps, core_ids=list(range(M)), **kwargs)
    global LAST
    LAST = res
    out = np.concatenate([res.results[m]["outp"] for m in range(M)], axis=0)
    return np.ascontiguousarray(out[:N])


LAST = None
TRACE = False
TRACE_DIR = None


# revision 19
# speedup vs baseline: 1.0999x; 1.0162x over previous
"""RGCN (2-layer, basis-decomposition) Trainium2 kernel, v4.

Strategy (8 NeuronCores, SPMD), building on v2/v3:
  - Edges sorted by destination; dst nodes partitioned into 8 contiguous
    ranges (one per core). Messages + segment-sum fused into per-tile PE
    matmuls against a DVE-built "weighted one-hot" (woh) matrix.
  - v3: layer-2 gather descriptors are PREPARED (prepare_only) on 4 SWDGE
    queues during layer 1, and each page's DMA is fired by a cheap
    trigger_dma once the AllGathered h table is ready; gather desc-gen is
    off the critical path.
  - v4: both layers use per-group cap+spill tiling (L1: 4 blocks of 32 x
    2 tiles + 1 spill tile per group; L2: 8 blocks of 16 x 1 tile + 1
    spill tile per group). Narrow blocks halve the DVE one-hot/multiply
    width per edge; spill tiles use a permuted group-wide one-hot that
    matches the (bi, basis, j) psum column layout.
"""

import math

import numpy as np
import ml_dtypes

import concourse.bacc as bacc
import concourse.bass as bass
import concourse.mybir as mybir
import concourse.tile as tile
from concourse.bass_utils import run_bass_kernel_spmd

F32 = mybir.dt.float32
BF16 = mybir.dt.bfloat16
I16 = mybir.dt.int16
AF = mybir.ActivationFunctionType
ALU = mybir.AluOpType
BF = ml_dtypes.bfloat16

M = 8            # cores
GRP = 128        # nodes per output group
TPE = 128        # edges per tile
GP = 36          # tiles per page (4 groups of 9)
CHK = 8          # base tiles per group (= one woh build chunk)
DW = 128         # padded table row width (256 B in bf16)
BLK1 = 32        # layer-1 block size
CAP1 = 2         # layer-1 base tiles per block
BLK2 = 16        # layer-2 block size
CAP2 = 1         # layer-2 base tiles per block
NCHUNK = 6       # AllGather chunks
GB_BUFS = 4      # resident layer-2 gather pages (WAR window)
NSUB = 4         # sub-preps per gather page
PREP_PAGES = 2   # pages desc-prepared during layer 1 (ring capacity bound)

TAIL_DELAY = 10   # chunks between a group's last matmul and its tail
TAIL_DELAY2 = 4   # layer-2 variant


def _expand(ap, free_dims, col_offset=0):
    """AP with the partition dim kept and explicit [step, count] free dims."""
    base = ap.ap
    return bass.AP(
        ap.tensor,
        ap.offset + col_offset,
        [list(base[0])] + [list(d) for d in free_dims],
    )


def _page_subranges(nt):
    base, rem = divmod(nt, NSUB)
    out, t = [], 0
    for i in range(NSUB):
        s = base + (1 if i < rem else 0)
        out.append((t, t + s))
        t += s
    return [r for r in out if r[1] > r[0]]


def _prep(N, D, entity, edge_index, edge_type, edge_norm, att1, att2):
    """Host-side graph preprocessing. Returns per-core arrays + structure."""
    NPC = int(math.ceil(N / (M * GRP))) * GRP      # nodes per core (6272)
    SEG = NPC + GRP                                # table segment per core
    NTAB = SEG * M                                 # 51200
    NGRP = NPC // GRP                              # 49

    # AllGather chunking of the 49 groups
    base_cg = NGRP // NCHUNK
    cg = [base_cg] * NCHUNK
    cg[0] += NGRP - base_cg * NCHUNK               # [13,12,12,12]
    chunk_of_group = np.repeat(np.arange(NCHUNK), cg)
    chunk_gstart = np.concatenate([[0], np.cumsum(cg)[:-1]])
    pr_c = [x * (GRP // 2) for x in cg]
    pr_c[0] += GRP // 2                            # zero region pair rows
    hfull_base = np.concatenate([[0], np.cumsum([p * M for p in pr_c])[:-1]])
    zero_row = int(hfull_base[0] + pr_c[0] - GRP // 2)

    src = np.asarray(edge_index[0], dtype=np.int64)
    dst = np.asarray(edge_index[1], dtype=np.int64)
    et = np.asarray(edge_type[:, 0], dtype=np.int64)
    norm = np.asarray(edge_norm, dtype=np.float32)

    order = np.argsort(dst, kind="stable")
    src_s, dst_s, et_s, norm_s = src[order], dst[order], et[order], norm[order]

    deg = np.bincount(dst, minlength=NPC * M).astype(np.float32)
    inv_deg = (1.0 / np.maximum(deg, 1.0)).astype(np.float32)
    core_of = dst_s // NPC

    def capspill(BLKx, CAP):
        """Per-group cap+spill tiling: per group bpg*CAP base tiles (one
        per (block, w)) + 1 spill tile; groups of 9 tiles, pages of GP."""
        bpg = GRP // BLKx
        nblk = NPC // BLKx
        tiles, chunks = [], []
        for g in range(NGRP):
            chunks.append(("s", 1, len(tiles)))
            tiles.append(("s", g, 0, 0))
            chunks.append(("b", bpg * CAP, len(tiles)))
            for bi in range(bpg):
                for w in range(CAP):
                    tiles.append(("b", g, bi, w))
        T = len(tiles)
        npage = -(-T // GP)
        Tpad = npage * GP
        page_tiles = [min(GP, T - p * GP) for p in range(npage)]
        base_tile_id = np.zeros((nblk, CAP), np.int64)
        spill_tile_id = np.zeros(NGRP, np.int64)
        for t, (kind, g, bi, w) in enumerate(tiles):
            if kind == "s":
                spill_tile_id[g] = t
            else:
                base_tile_id[g * bpg + bi, w] = t
        percore = []
        for m in range(M):
            eidx = np.nonzero(core_of == m)[0]
            dl = dst_s[eidx] - m * NPC
            blk = dl // BLKx
            es = np.zeros(nblk + 1, np.int64)
            es[1:] = np.cumsum(np.bincount(blk, minlength=nblk))
            w2 = np.arange(len(eidx)) - es[blk]
            is_base = w2 < CAP * TPE
            sl = np.empty(len(eidx), np.int64)
            sl[is_base] = (base_tile_id[blk[is_base], w2[is_base] // TPE]
                           * TPE + w2[is_base] % TPE)
            sp = ~is_base
            sp_cnt = np.maximum(0, np.diff(es) - CAP * TPE)
            g_of = np.arange(nblk) // bpg
            tot = np.cumsum(sp_cnt) - sp_cnt
            first = np.searchsorted(g_of, np.arange(NGRP))
            sp_base = tot - tot[first][g_of]
            sw = w2[sp] - CAP * TPE + sp_base[blk[sp]]
            assert sw.max(initial=0) < TPE, ("spill overflow",
                                             int(sw.max(initial=0)))
            sl[sp] = spill_tile_id[g_of[blk[sp]]] * TPE + sw
            do = np.where(is_base, dl % BLKx, dl % GRP).astype(np.float32)
            percore.append((eidx, sl, do))
        return dict(chunks=chunks, tiles=tiles, T=T, Tpad=Tpad,
                    NPAGE=npage, page_tiles=page_tiles, percore=percore,
                    bpg=bpg)

    L1 = capspill(BLK1, CAP1)
    L2 = capspill(BLK2, CAP2)

    nd = norm_s * inv_deg[dst_s]
    c1 = (att1[et_s] * nd[:, None]).astype(np.float32)
    c2 = (att2[et_s] * nd[:, None]).astype(np.float32)

    # layer-2 gather: chunk-major hfull pair-row index for each source node
    k_s = src_s // NPC
    n_s = src_s % NPC
    g_s = n_s // GRP
    c_s = chunk_of_group[g_s]
    lpr = (g_s - chunk_gstart[c_s]) * (GRP // 2) + (n_s % GRP) // 2
    row_s = hfull_base[c_s] + k_s * np.asarray(pr_c)[c_s] + lpr
    par_s = (n_s % 2).astype(np.float32)

    def pack_idx(lin, npage):
        """[npage*GP*TPE] indices -> [npage, 128, GP*8] int16 layout."""
        out = np.empty((npage, 16, GP * 8), np.int16)
        lp = lin.reshape(npage, GP * TPE)
        out[:, :, :] = lp.reshape(npage, GP * 8, 16).transpose(0, 2, 1)
        return np.ascontiguousarray(np.tile(out, (1, 8, 1)))

    def scat(vals, sl, Tpad, fill=0.0, width=None):
        if width is None:
            out = np.full(Tpad * TPE, fill, dtype=np.float32)
        else:
            out = np.full((Tpad * TPE, width), fill, dtype=np.float32)
        out[sl] = vals
        return out

    def tilemaj(a, npage, w):
        """[Tpad*TPE(,w)] slot-major -> [npage, TPE, GP*w] tile-major."""
        if a.ndim == 1:
            a = a[:, None]
        return np.ascontiguousarray(
            a.reshape(npage, GP, TPE, a.shape[1]).transpose(0, 2, 1, 3)
            .reshape(npage, TPE, GP * a.shape[1]).astype(BF))

    ent = np.asarray(entity, dtype=np.float32)

    metac1s, metac2s, idx_pages, xpgs = [], [], [], []
    for m in range(M):
        eidx, sl1, do1 = L1["percore"][m]
        NP1, TP1 = L1["NPAGE"], L1["Tpad"]
        xsrcm = np.zeros((TP1 * TPE, D), dtype=BF)
        xsrcm[sl1] = ent[src_s[eidx]].astype(BF)
        xpgs.append(tilemaj(xsrcm.astype(np.float32), NP1, D))
        metac1 = np.concatenate([
            tilemaj(scat(do1, sl1, TP1, 320.0), NP1, 1),
            tilemaj(scat(c1[eidx], sl1, TP1, width=4), NP1, 4)], axis=2)
        metac1s.append(np.ascontiguousarray(metac1))

        eidx2, sl2, do2 = L2["percore"][m]
        NP2, TP2 = L2["NPAGE"], L2["Tpad"]
        rows = np.full(TP2 * TPE, zero_row, dtype=np.int64)
        rows[sl2] = row_s[eidx2]
        idx_pages.append(pack_idx(rows.astype(np.int16), NP2))
        par = scat(par_s[eidx2], sl2, TP2)
        ca = scat(c2[eidx2], sl2, TP2, width=4)
        metac2 = np.concatenate([
            tilemaj(scat(do2, sl2, TP2, 320.0), NP2, 1),
            tilemaj(ca * (1.0 - par)[:, None], NP2, 4),
            tilemaj(ca * par[:, None], NP2, 4)], axis=2)
        metac2s.append(np.ascontiguousarray(metac2))

    return dict(NPC=NPC, SEG=SEG, NTAB=NTAB, NGRP=NGRP, L1=L1, L2=L2,
                cg=cg, chunk_of_group=chunk_of_group,
                chunk_gstart=chunk_gstart, pr_c=pr_c, hfull_base=hfull_base,
                idxp=idx_pages, metac1=metac1s, metac2=metac2s, xpg=xpgs)


def _chunks_by_page(L):
    out = [[] for _ in range(L["NPAGE"])]
    for kind, chks, toff in L["chunks"]:
        out[toff // GP].append((kind, chks, toff))
    return out


def _make_prepper(nc, prm, gath2, table_ap, idxt_all, gsems, psems):
    """Prepare-only gather desc-gen for the first PREP_PAGES layer-2 pages.

    Each page's NSUB sub-preps are spread round-robin across the 4 SWDGE
    queues (keeps parked-descriptor count per ring low and lets the 4 Q7
    core pairs generate concurrently). trigger(q) fires page q's subs
    (count=1 on each queue, FIFO order); gwait(q) gates the tensor engine
    on the page's gather DMAs. Later pages use plain direct gathers."""
    page_tiles = prm["L2"]["page_tiles"]
    gbufs = {}
    nsub_q = [0, 0, 0, 0]       # subs emitted per queue
    ntrig_q = [0, 0, 0, 0]      # subs triggered per queue

    def emit_page_preps(q):
        if q not in gbufs:
            gbufs[q] = gath2.tile([TPE, GP, 2 * DW], BF16, tag="gbuf2",
                                  bufs=GB_BUFS, name="gbuf2")
        for si, (t0, t1) in enumerate(_page_subranges(page_tiles[q])):
            qq = si % 4
            nidx = (t1 - t0) * TPE
            nc.gpsimd.dma_gather(
                out_ap=gbufs[q][:, t0:t1, :], in_ap=table_ap,
                idxs_ap=idxt_all[:, q, t0 * 8:t1 * 8],
                num_idxs=nidx, num_idxs_reg=nidx,
                elem_size=2 * DW, single_packet=False, queue_num=qq,
                prepare_only=True, sem=gsems[qq]).then_inc(psems[qq], 1)
            nsub_q[qq] += 1

    def emit_direct(q):
        qq = q % 4
        if q not in gbufs:
            gbufs[q] = gath2.tile([TPE, GP, 2 * DW], BF16, tag="gbuf2",
                                  bufs=GB_BUFS, name="gbuf2")
        nt = page_tiles[q]
        nidx = nt * TPE
        nc.gpsimd.dma_gather(
            out_ap=gbufs[q][:, 0:nt, :], in_ap=table_ap,
            idxs_ap=idxt_all[:, q, 0:nt * 8],
            num_idxs=nidx, num_idxs_reg=nidx,
            elem_size=2 * DW, single_packet=False, queue_num=qq)

    def trigger(q):
        nsubs = len(_page_subranges(page_tiles[q]))
        for si in range(nsubs):
            qq = si % 4
            ntrig_q[qq] += 1
            nc.gpsimd.wait_ge(psems[qq], ntrig_q[qq])
            nc.gpsimd.trigger_dma(count=1, queue_num=qq)

    def gwait(q):
        for qq in range(4):
            nc.tensor.wait_ge(gsems[qq], 16 * (q + 1))

    return emit_page_preps, trigger, gbufs, emit_direct, gwait


def _layer1(tc, nc, pools, prm, D, xpg, metac1_all, iota1_t, iota128_t,
            ident_t, bas_t, rt_t, bias_t, xslb, h4buf, hsl_cs, group_done,
            prep_hook):
    """Dense layer 1: pre-gathered x rows; per-group spill + base tiles."""
    gath, wohp, hp, xtp, sbigp, psp = pools
    L = prm["L1"]
    NPAGE, tiles, bpg = L["NPAGE"], L["tiles"], L["bpg"]
    cbp = _chunks_by_page(L)

    pending = []

    def flush(now):
        while pending and (now is None or pending[0][0] + TAIL_DELAY <= now):
            _, g, sbig = pending.pop(0)
            _tail1(tc, nc, pools, prm, D, g, sbig, xslb, h4buf,
                   ident_t, bas_t, rt_t, bias_t, hsl_cs)
            group_done(g)
            prep_hook()

    psums = {}
    cglob = 0
    for q in range(NPAGE):
        gbuf = gath.tile([TPE, GP * D], BF16, tag="gbuf1", bufs=3)
        nc.sync.dma_start(out=gbuf[:], in_=xpg[q])
        metat = metac1_all[:, q, :]
        for kind, chks, toff in cbp[q]:
            flush(cglob)
            cglob += 1
            loff = toff - q * GP
            if kind == "s":
                g = tiles[toff][1]
                ohs = wohp.tile([TPE, GRP], BF16, tag="ohs1", bufs=3)
                nc.vector.tensor_tensor(
                    out=ohs[:], in0=iota128_t,
                    in1=_expand(metat, [[0, GRP]], col_offset=loff),
                    op=ALU.is_equal)
                wohs = wohp.tile([TPE, 4 * GRP], BF16, tag="wohs1", bufs=3)
                for b in range(4):
                    nc.vector.tensor_tensor(
                        out=_expand(wohs[:], [[4 * BLK1, bpg], [1, BLK1]],
                                    col_offset=b * BLK1),
                        in0=_expand(ohs[:], [[BLK1, bpg], [1, BLK1]]),
                        in1=_expand(metat, [[0, bpg], [0, BLK1]],
                                    col_offset=GP + loff * 4 + b),
                        op=ALU.mult)
                psum_blk = psp.tile([D, 4 * GRP], F32, tag="blk", bufs=3,
                                    name="psum_blk")
                psums[g] = psum_blk
                nc.tensor.matmul(out=psum_blk[:],
                                 lhsT=gbuf[:, loff * D:(loff + 1) * D],
                                 rhs=wohs[:], start=True, stop=False)
            else:
                oh8 = wohp.tile([TPE, CHK * BLK1], BF16, tag="oh1", bufs=4)
                nc.vector.tensor_tensor(
                    out=_expand(oh8[:], [[BLK1, CHK], [1, BLK1]]),
                    in0=iota1_t,
                    in1=_expand(metat, [[1, CHK], [0, BLK1]],
                                col_offset=loff),
                    op=ALU.is_equal)
                woh = wohp.tile([TPE, CHK * 4 * BLK1], BF16, tag="woh1",
                                bufs=4)
                for b in range(4):
                    nc.vector.tensor_tensor(
                        out=_expand(woh[:], [[4 * BLK1, CHK], [1, BLK1]],
                                    col_offset=b * BLK1),
                        in0=_expand(oh8[:], [[BLK1, CHK], [1, BLK1]]),
                        in1=_expand(metat, [[4, CHK], [0, BLK1]],
                                    col_offset=GP + loff * 4 + b),
                        op=ALU.mult)
                for u in range(CHK):
                    _, g, bi, w = tiles[toff + u]
                    last = (bi == bpg - 1 and w == CAP1 - 1)
                    nc.tensor.matmul(
                        out=psums[g][:, bi * 4 * BLK1:(bi + 1) * 4 * BLK1],
                        lhsT=gbuf[:, (loff + u) * D:(loff + u + 1) * D],
                        rhs=woh[:, u * 4 * BLK1:(u + 1) * 4 * BLK1],
                        start=False, stop=last)
                    if last:
                        psum_blk = psums.pop(g)
                        sbig = sbigp.tile([D, 4 * GRP], BF16, tag="sbig",
                                          bufs=5)
                        for b in range(4):
                            nc.scalar.copy(
                                out=_expand(sbig[:], [[BLK1, bpg], [1, BLK1]],
                                            col_offset=b * GRP),
                                in_=_expand(psum_blk[:],
                                            [[4 * BLK1, bpg], [1, BLK1]],
                                            col_offset=b * BLK1))
                        pending.append((cglob, g, sbig))
    flush(None)


def _tail1(tc, nc, pools, prm, D, g, sbig, xslb, h4buf, ident_t,
           bas_t, rt_t, bias_t, hsl_cs):
    """Layer-1 group tail: combine bases, mean, root, bias, relu -> hsl."""
    gath, wohp, hp, xtp, sbigp, psp = pools
    pagg = psp.tile([GRP, D], F32, tag="agg", bufs=2)
    for b in range(4):
        nc.tensor.matmul(
            out=pagg[:],
            lhsT=sbig[:, b * GRP:(b + 1) * GRP],
            rhs=bas_t[:, b * D:(b + 1) * D],
            start=(b == 0),
            stop=False,
        )
    nc.tensor.matmul(out=pagg[:], lhsT=xslb[:, g, :], rhs=rt_t,
                     start=False, stop=True)

    h3 = hp.tile([GRP, D], F32, tag="h3")
    nc.vector.tensor_tensor(out=h3[:], in0=pagg[:], in1=bias_t, op=ALU.add)

    nc.scalar.activation(out=h4buf[:, g, 0:D], in_=h3[:], func=AF.Relu)
    c = int(prm["chunk_of_group"][g])
    gl = g - int(prm["chunk_gstart"][c])
    base = hsl_cs[c][:]
    dst = bass.AP(base.tensor, base.offset + gl * (GRP // 2) * (2 * DW),
                  [[2 * DW, GRP // 2], [DW, 2], [1, DW]])
    nc.sync.dma_start(out=dst, in_=h4buf[:, g, :])


def _layer2(tc, nc, pools, prm, D, metat_all,
            iota2_t, iota128_t, ident_t, bas_t, rt_t, bias_t, h4buf,
            out_ap, trigger, gbufs, gwait, emit_direct):
    """Gather-based layer 2: per-group spill + 8 base tiles of 16."""
    gath, wohp, hp, xtp, sbigp, psp = pools
    L = prm["L2"]
    NPAGE, tiles, bpg = L["NPAGE"], L["tiles"], L["bpg"]
    cbp = _chunks_by_page(L)

    pending = []

    def flush(now):
        while pending and (now is None or pending[0][0] + TAIL_DELAY2 <= now):
            _, g, sbig = pending.pop(0)
            _tail2(tc, nc, pools, prm, D, g, sbig, h4buf, ident_t,
                   bas_t, rt_t, bias_t, out_ap)

    for q in range(min(GB_BUFS, NPAGE)):
        emit_direct(q)

    psums = {}
    cglob = 0
    for q in range(NPAGE):
        if q > 0 and q - 1 + GB_BUFS < NPAGE:
            emit_direct(q - 1 + GB_BUFS)
        metat = metat_all[:, q, :]
        gbuf = gbufs[q]
        for kind, chks, toff in cbp[q]:
            flush(cglob)
            cglob += 1
            loff = toff - q * GP
            if kind == "s":
                g = tiles[toff][1]
                ohs = wohp.tile([TPE, GRP], BF16, tag="ohs2", bufs=3)
                nc.vector.tensor_tensor(
                    out=ohs[:], in0=iota128_t,
                    in1=_expand(metat, [[0, GRP]], col_offset=loff),
                    op=ALU.is_equal)
                wohsE = wohp.tile([TPE, 4 * GRP], BF16, tag="wohsE", bufs=3)
                wohsO = wohp.tile([TPE, 4 * GRP], BF16, tag="wohsO", bufs=3)
                for woh, cbase in ((wohsE, GP), (wohsO, 5 * GP)):
                    for b in range(4):
                        nc.vector.tensor_tensor(
                            out=_expand(woh[:], [[4 * BLK2, bpg], [1, BLK2]],
                                        col_offset=b * BLK2),
                            in0=_expand(ohs[:], [[BLK2, bpg], [1, BLK2]]),
                            in1=_expand(metat, [[0, bpg], [0, BLK2]],
                                        col_offset=cbase + loff * 4 + b),
                            op=ALU.mult)
                psum_blk = psp.tile([D, 4 * GRP], F32, tag="blk", bufs=3,
                                    name="psum_blk")
                psums[g] = psum_blk
                nc.tensor.matmul(out=psum_blk[:], lhsT=gbuf[:, loff, 0:D],
                                 rhs=wohsE[:], start=True, stop=False)
                nc.tensor.matmul(out=psum_blk[:],
                                 lhsT=gbuf[:, loff, DW:DW + D],
                                 rhs=wohsO[:], start=False, stop=False)
            else:
                oh8 = wohp.tile([TPE, CHK * BLK2], BF16, tag="oh2", bufs=5)
                nc.vector.tensor_tensor(
                    out=_expand(oh8[:], [[BLK2, CHK], [1, BLK2]]),
                    in0=iota2_t,
                    in1=_expand(metat, [[1, CHK], [0, BLK2]],
                                col_offset=loff),
                    op=ALU.is_equal)
                wohE = wohp.tile([TPE, CHK * 4 * BLK2], BF16, tag="wohE",
                                 bufs=4)
                wohO = wohp.tile([TPE, CHK * 4 * BLK2], BF16, tag="wohO",
                                 bufs=4)
                for woh, cbase in ((wohE, GP), (wohO, 5 * GP)):
                    for b in range(4):
                        nc.vector.tensor_tensor(
                            out=_expand(woh[:], [[4 * BLK2, CHK], [1, BLK2]],
                                        col_offset=b * BLK2),
                            in0=_expand(oh8[:], [[BLK2, CHK], [1, BLK2]]),
                            in1=_expand(metat, [[4, CHK], [0, BLK2]],
                                        col_offset=cbase + loff * 4 + b),
                            op=ALU.mult)
                for u in range(CHK):
                    _, g, bi, w = tiles[toff + u]
                    stop = (bi == bpg - 1)
                    out_sl = psums[g][:, bi * 4 * BLK2:(bi + 1) * 4 * BLK2]
                    nc.tensor.matmul(out=out_sl,
                                     lhsT=gbuf[:, loff + u, 0:D],
                                     rhs=wohE[:, u * 4 * BLK2:(u + 1) * 4 * BLK2],
                                     start=False, stop=False)
                    nc.tensor.matmul(out=out_sl,
                                     lhsT=gbuf[:, loff + u, DW:DW + D],
                                     rhs=wohO[:, u * 4 * BLK2:(u + 1) * 4 * BLK2],
                                     start=False, stop=stop)
                    if stop:
                        psum_blk = psums.pop(g)
                        sbig = sbigp.tile([D, 4 * GRP], BF16, tag="sbig",
                                          bufs=5)
                        for b in range(4):
                            nc.scalar.copy(
                                out=_expand(sbig[:], [[BLK2, bpg], [1, BLK2]],
                                            col_offset=b * GRP),
                                in_=_expand(psum_blk[:],
                                            [[4 * BLK2, bpg], [1, BLK2]],
                                            col_offset=b * BLK2))
                        pending.append((cglob, g, sbig))
    flush(None)


def _tail2(tc, nc, pools, prm, D, g, sbig, h4buf, ident_t,
           bas_t, rt_t, bias_t, out_ap):
    gath, wohp, hp, xtp, sbigp, psp = pools
    ptr = psp.tile([D, GRP], BF16, tag="tr", bufs=2)
    nc.tensor.transpose(out=ptr[:], in_=h4buf[:, g, 0:D], identity=ident_t)
    xt = xtp.tile([D, GRP], BF16, tag="xt")
    nc.scalar.copy(out=xt[:], in_=ptr[:])

    pagg = psp.tile([GRP, D], F32, tag="agg", bufs=2)
    for b in range(4):
        nc.tensor.matmul(
            out=pagg[:],
            lhsT=sbig[:, b * GRP:(b + 1) * GRP],
            rhs=bas_t[:, b * D:(b + 1) * D],
            start=(b == 0),
            stop=False,
        )
    nc.tensor.matmul(out=pagg[:], lhsT=xt[:], rhs=rt_t, start=False, stop=True)

    h3 = hp.tile([GRP, D], F32, tag="h3")
    nc.vector.tensor_tensor(out=h3[:], in0=pagg[:], in1=bias_t, op=ALU.add)
    nc.scalar.dma_start(out=out_ap[g * GRP:(g + 1) * GRP, :], in_=h3[:])


def _build(prm, D):
    NPC, NTAB = prm["NPC"], prm["NTAB"]
    NGRP = prm["NGRP"]
    NPAGE1, NPAGE2 = prm["L1"]["NPAGE"], prm["L2"]["NPAGE"]
    pr_c, cg = prm["pr_c"], prm["cg"]
    nc = bacc.Bacc(num_swdge_queues=4)

    xslp = nc.dram_tensor("xslp", [D, NGRP * GRP], BF16, kind="ExternalInput")
    xpg = nc.dram_tensor("xpg", [NPAGE1, TPE, GP * D], BF16,
                         kind="ExternalInput")
    idxp = nc.dram_tensor("idxp", [NPAGE2, TPE, 8 * GP], I16,
                          kind="ExternalInput")
    metac1 = nc.dram_tensor("metac1", [NPAGE1, TPE, 5 * GP], BF16,
                            kind="ExternalInput")
    metac2 = nc.dram_tensor("metac2", [NPAGE2, TPE, 9 * GP], BF16,
                            kind="ExternalInput")
    iota1 = nc.dram_tensor("iota1", [TPE, CHK * BLK1], BF16,
                           kind="ExternalInput")
    iota2 = nc.dram_tensor("iota2", [TPE, CHK * BLK2], BF16,
                           kind="ExternalInput")
    iota128 = nc.dram_tensor("iota128", [TPE, GRP], BF16,
                             kind="ExternalInput")
    ident = nc.dram_tensor("ident", [TPE, TPE], BF16, kind="ExternalInput")
    bas1 = nc.dram_tensor("bas1", [D, 4 * D], BF16, kind="ExternalInput")
    bas2 = nc.dram_tensor("bas2", [D, 4 * D], BF16, kind="ExternalInput")
    rt1 = nc.dram_tensor("rt1", [D, D], BF16, kind="ExternalInput")
    rt2 = nc.dram_tensor("rt2", [D, D], BF16, kind="ExternalInput")
    bias1 = nc.dram_tensor("bias1", [GRP, D], F32, kind="ExternalInput")
    bias2 = nc.dram_tensor("bias2", [GRP, D], F32, kind="ExternalInput")
    outp = nc.dram_tensor("outp", [NPC, D], F32, kind="ExternalOutput")

    with tile.TileContext(nc) as tc:
        with (
            tc.tile_pool(name="const", bufs=1) as cst,
            tc.tile_pool(name="gath", bufs=3) as gath,
            tc.tile_pool(name="woh", bufs=3) as wohp,
            tc.tile_pool(name="hp", bufs=3) as hp,
            tc.tile_pool(name="xtp", bufs=3) as xtp,
            tc.tile_pool(name="sbig", bufs=2) as sbigp,
            tc.tile_pool(name="ps", bufs=1, space="PSUM") as psp,
            tc.tile_pool(name="dram", bufs=1, space="DRAM") as dramp,
        ):
            pools = (gath, wohp, hp, xtp, sbigp, psp)

            iota1_t = cst.tile([TPE, CHK * BLK1], BF16)
            nc.sync.dma_start(out=iota1_t[:], in_=iota1[:])
            iota2_t = cst.tile([TPE, CHK * BLK2], BF16)
            nc.sync.dma_start(out=iota2_t[:], in_=iota2[:])
            iota128_t = cst.tile([TPE, GRP], BF16)
            nc.sync.dma_start(out=iota128_t[:], in_=iota128[:])
            xslb = cst.tile([D, NGRP, GRP], BF16)
            nc.sync.dma_start(out=xslb[:], in_=xslp[:])
            h4buf = cst.tile([GRP, NGRP, DW], BF16)
            nc.vector.memset(h4buf[:], 0.0)
            ident_t = cst.tile([TPE, TPE], BF16)
            nc.sync.dma_start(out=ident_t[:], in_=ident[:])
            bas1_t = cst.tile([D, 4 * D], BF16)
            nc.sync.dma_start(out=bas1_t[:], in_=bas1[:])
            bas2_t = cst.tile([D, 4 * D], BF16)
            nc.sync.dma_start(out=bas2_t[:], in_=bas2[:])
            rt1_t = cst.tile([D, D], BF16)
            nc.sync.dma_start(out=rt1_t[:], in_=rt1[:])
            rt2_t = cst.tile([D, D], BF16)
            nc.sync.dma_start(out=rt2_t[:], in_=rt2[:])
            bias1_t = cst.tile([GRP, D], F32)
            nc.sync.dma_start(out=bias1_t[:], in_=bias1[:])
            bias2_t = cst.tile([GRP, D], F32)
            nc.sync.dma_start(out=bias2_t[:], in_=bias2[:])
            zed_t = cst.tile([GRP, 2 * DW], BF16)
            nc.gpsimd.memset(zed_t[:], 0.0)

            # preloaded layer-2 gather indices + both layers' meta pages
            idxt_all = cst.tile([TPE, NPAGE2, 8 * GP], I16)
            ib = idxp[:]
            nc.sync.dma_start(
                out=idxt_all[:],
                in_=bass.AP(ib.tensor, ib.offset,
                            [[8 * GP, TPE], [TPE * 8 * GP, NPAGE2],
                             [1, 8 * GP]]))
            metat2_all = cst.tile([TPE, NPAGE2, 9 * GP], BF16)
            mb2 = metac2[:]
            nc.sync.dma_start(
                out=metat2_all[:],
                in_=bass.AP(mb2.tensor, mb2.offset,
                            [[9 * GP, TPE], [TPE * 9 * GP, NPAGE2],
                             [1, 9 * GP]]))
            metac1_all = cst.tile([TPE, NPAGE1, 5 * GP], BF16)
            mb1 = metac1[:]
            nc.sync.dma_start(
                out=metac1_all[:],
                in_=bass.AP(mb1.tensor, mb1.offset,
                            [[5 * GP, TPE], [TPE * 5 * GP, NPAGE1],
                             [1, 5 * GP]]))

            hsl_cs = [dramp.tile([pr_c[c], 2 * DW], BF16, name=f"hsl_c{c}")
                      for c in range(NCHUNK)]
            hfull = dramp.tile([NTAB // 2, 2 * DW], BF16)

            # zero rows at the tail of chunk 0 (dummy gather targets)
            nc.sync.dma_start(
                out=hsl_cs[0][pr_c[0] - GRP // 2:pr_c[0], :],
                in_=zed_t[0:GRP // 2, :])

            gsems = [nc.alloc_semaphore(f"gsem{q}") for q in range(4)]
            psems = [nc.alloc_semaphore(f"psem{q}") for q in range(4)]
            for s in gsems + psems:
                nc.gpsimd.sem_clear(s)
            hchk = cst.tile([1, 2 * DW], BF16)
            hchk2 = cst.tile([1, 2 * DW], BF16)

            fired = [False] * NCHUNK
            last_group_of_chunk = np.cumsum(cg) - 1

            def group_done(g):
                for c in range(NCHUNK):
                    if g == last_group_of_chunk[c] and not fired[c]:
                        fired[c] = True
                        base = int(prm["hfull_base"][c] // 1)
                        rows = pr_c[c] * M
                        nc.gpsimd.collective_compute(
                            "AllGather",
                            ALU.bypass,
                            replica_groups=[list(range(M))],
                            ins=[hsl_cs[c][:]],
                            outs=[hfull[base:base + rows, :]],
                        )

            with tc.tile_pool(name="gath2", bufs=GB_BUFS) as gath2:
                (emit_page_preps, trigger, gbufs, emit_direct,
                 gwait) = _make_prepper(
                    nc, prm, gath2, hfull[:, :], idxt_all, gsems, psems)

                prep_q = []

                def prep_hook():
                    pass

                _layer1(tc, nc, pools, prm, D, xpg, metac1_all,
                        iota1_t[:], iota128_t[:], ident_t[:], bas1_t,
                        rt1_t[:], bias1_t[:], xslb, h4buf, hsl_cs,
                        group_done, prep_hook)

                _layer2(tc, nc, pools, prm, D, metat2_all,
                        iota2_t[:], iota128_t[:], ident_t[:], bas2_t,
                        rt2_t[:], bias2_t[:], h4buf, outp,
                        trigger, gbufs, gwait, emit_direct)
    nc.compile()
    return nc


def kernel(entity, edge_index, edge_attr, edge_type, edge_norm,
           basis1, att1, root1, bias1, basis2, att2, root2, bias2):
    N, D = entity.shape
    prm = _prep(N, D, np.asarray(entity), np.asarray(edge_index),
                np.asarray(edge_type), np.asarray(edge_norm),
                np.asarray(att1), np.asarray(att2))
    NPC = prm["NPC"]

    entity = np.asarray(entity, dtype=np.float32)

    iota1_arr = np.tile(np.arange(BLK1, dtype=np.float32), (TPE, CHK)).astype(BF)
    iota2_arr = np.tile(np.arange(BLK2, dtype=np.float32), (TPE, CHK)).astype(BF)
    iota128_arr = np.tile(np.arange(GRP, dtype=np.float32), (TPE, 1)).astype(BF)
    ident_arr = np.eye(TPE, dtype=np.float32).astype(BF)
    b1 = np.ascontiguousarray(
        np.asarray(basis1, np.float32).transpose(1, 0, 2).reshape(D, 4 * D)).astype(BF)
    b2 = np.ascontiguousarray(
        np.asarray(basis2, np.float32).transpose(1, 0, 2).reshape(D, 4 * D)).astype(BF)

    nc = _build(prm, D)

    in_maps = []
    for m in range(M):
        lo, hi = m * NPC, min((m + 1) * NPC, N)
        xs = np.zeros((NPC, D), dtype=np.float32)
        if hi > lo:
            xs[0:hi - lo] = entity[lo:hi]
        xslp_arr = np.ascontiguousarray(
            xs.reshape(prm["NGRP"], GRP, D).transpose(2, 0, 1)
            .reshape(D, prm["NGRP"] * GRP)).astype(BF)
        in_maps.append({
            "xslp": xslp_arr,
            "xpg": prm["xpg"][m],
            "idxp": prm["idxp"][m],
            "metac1": prm["metac1"][m],
            "metac2": prm["metac2"][m],
            "iota1": iota1_arr,
            "iota2": iota2_arr,
            "iota128": iota128_arr,
            "ident": ident_arr,
            "bas1": b1,
            "bas2": b2,
            "rt1": np.asarray(root1, np.float32).astype(BF),
            "rt2": np.asarray(root2, np.float32).astype(BF),
            "bias1": np.tile(np.asarray(bias1, np.float32), (GRP, 1)),
            "bias2": np.tile(np.asarray(bias2, np.float32), (GRP, 1)),
        })
    kwargs = {}
    if TRACE:
        kwargs = dict(trace=True, tmpdir=TRACE_DIR)
    res = run_bass_kernel_spmd(nc, in_maps, core_ids=list(range(M)), **kwargs)
    global LAST
    LAST = res
    out = np.concatenate([res.results[m]["outp"] for m in range(M)], axis=0)
    return np.ascontiguousarray(out[:N])


LAST = None
TRACE = False
TRACE_DIR = None


# revision 20
# speedup vs baseline: 1.1879x; 1.0800x over previous
"""RGCN (2-layer, basis-decomposition) Trainium2 kernel, v4.

Strategy (8 NeuronCores, SPMD), building on v2/v3:
  - Edges sorted by destination; dst nodes partitioned into 8 contiguous
    ranges (one per core). Messages + segment-sum fused into per-tile PE
    matmuls against a DVE-built "weighted one-hot" (woh) matrix.
  - v3: layer-2 gather descriptors are PREPARED (prepare_only) on 4 SWDGE
    queues during layer 1, and each page's DMA is fired by a cheap
    trigger_dma once the AllGathered h table is ready; gather desc-gen is
    off the critical path.
  - v4: both layers use per-group cap+spill tiling (L1: 4 blocks of 32 x
    2 tiles + 1 spill tile per group; L2: 8 blocks of 16 x 1 tile + 1
    spill tile per group). Narrow blocks halve the DVE one-hot/multiply
    width per edge; spill tiles use a permuted group-wide one-hot that
    matches the (bi, basis, j) psum column layout.
"""

import math

import numpy as np
import ml_dtypes

import concourse.bacc as bacc
import concourse.bass as bass
import concourse.mybir as mybir
import concourse.tile as tile
from concourse.bass_utils import run_bass_kernel_spmd

F32 = mybir.dt.float32
BF16 = mybir.dt.bfloat16
I16 = mybir.dt.int16
AF = mybir.ActivationFunctionType
ALU = mybir.AluOpType
BF = ml_dtypes.bfloat16

M = 8            # cores
GRP = 128        # nodes per output group
TPE = 128        # edges per tile
GP = 36          # tiles per page (4 groups of 9)
CHK = 8          # base tiles per group (= one woh build chunk)
DW = 128         # padded table row width (256 B in bf16)
BLK1 = 16        # layer-1 block size
CAP1 = 1         # layer-1 base tiles per block
BLK2 = 16        # layer-2 block size
CAP2 = 1         # layer-2 base tiles per block
NCHUNK = 6       # AllGather chunks
GB_BUFS = 4      # resident layer-2 gather pages (WAR window)
NSUB = 4         # sub-preps per gather page
PREP_PAGES = 2   # pages desc-prepared during layer 1 (ring capacity bound)

TAIL_DELAY = 10   # chunks between a group's last matmul and its tail
TAIL_DELAY2 = 4   # layer-2 variant


def _expand(ap, free_dims, col_offset=0):
    """AP with the partition dim kept and explicit [step, count] free dims."""
    base = ap.ap
    return bass.AP(
        ap.tensor,
        ap.offset + col_offset,
        [list(base[0])] + [list(d) for d in free_dims],
    )


def _page_subranges(nt):
    base, rem = divmod(nt, NSUB)
    out, t = [], 0
    for i in range(NSUB):
        s = base + (1 if i < rem else 0)
        out.append((t, t + s))
        t += s
    return [r for r in out if r[1] > r[0]]


def _prep(N, D, entity, edge_index, edge_type, edge_norm, att1, att2):
    """Host-side graph preprocessing. Returns per-core arrays + structure."""
    NPC = int(math.ceil(N / (M * GRP))) * GRP      # nodes per core (6272)
    SEG = NPC + GRP                                # table segment per core
    NTAB = SEG * M                                 # 51200
    NGRP = NPC // GRP                              # 49

    # AllGather chunking of the 49 groups
    base_cg = NGRP // NCHUNK
    cg = [base_cg] * NCHUNK
    cg[0] += NGRP - base_cg * NCHUNK               # [13,12,12,12]
    chunk_of_group = np.repeat(np.arange(NCHUNK), cg)
    chunk_gstart = np.concatenate([[0], np.cumsum(cg)[:-1]])
    pr_c = [x * (GRP // 2) for x in cg]
    pr_c[0] += GRP // 2                            # zero region pair rows
    hfull_base = np.concatenate([[0], np.cumsum([p * M for p in pr_c])[:-1]])
    zero_row = int(hfull_base[0] + pr_c[0] - GRP // 2)

    src = np.asarray(edge_index[0], dtype=np.int64)
    dst = np.asarray(edge_index[1], dtype=np.int64)
    et = np.asarray(edge_type[:, 0], dtype=np.int64)
    norm = np.asarray(edge_norm, dtype=np.float32)

    order = np.argsort(dst, kind="stable")
    src_s, dst_s, et_s, norm_s = src[order], dst[order], et[order], norm[order]

    deg = np.bincount(dst, minlength=NPC * M).astype(np.float32)
    inv_deg = (1.0 / np.maximum(deg, 1.0)).astype(np.float32)
    core_of = dst_s // NPC

    def capspill(BLKx, CAP):
        """Per-group cap+spill tiling: per group bpg*CAP base tiles (one
        per (block, w)) + 1 spill tile; groups of 9 tiles, pages of GP."""
        bpg = GRP // BLKx
        nblk = NPC // BLKx
        tiles, chunks = [], []
        for g in range(NGRP):
            chunks.append(("s", 1, len(tiles)))
            tiles.append(("s", g, 0, 0))
            chunks.append(("b", bpg * CAP, len(tiles)))
            for bi in range(bpg):
                for w in range(CAP):
                    tiles.append(("b", g, bi, w))
        T = len(tiles)
        npage = -(-T // GP)
        Tpad = npage * GP
        page_tiles = [min(GP, T - p * GP) for p in range(npage)]
        base_tile_id = np.zeros((nblk, CAP), np.int64)
        spill_tile_id = np.zeros(NGRP, np.int64)
        for t, (kind, g, bi, w) in enumerate(tiles):
            if kind == "s":
                spill_tile_id[g] = t
            else:
                base_tile_id[g * bpg + bi, w] = t
        percore = []
        for m in range(M):
            eidx = np.nonzero(core_of == m)[0]
            dl = dst_s[eidx] - m * NPC
            blk = dl // BLKx
            es = np.zeros(nblk + 1, np.int64)
            es[1:] = np.cumsum(np.bincount(blk, minlength=nblk))
            w2 = np.arange(len(eidx)) - es[blk]
            is_base = w2 < CAP * TPE
            sl = np.empty(len(eidx), np.int64)
            sl[is_base] = (base_tile_id[blk[is_base], w2[is_base] // TPE]
                           * TPE + w2[is_base] % TPE)
            sp = ~is_base
            sp_cnt = np.maximum(0, np.diff(es) - CAP * TPE)
            g_of = np.arange(nblk) // bpg
            tot = np.cumsum(sp_cnt) - sp_cnt
            first = np.searchsorted(g_of, np.arange(NGRP))
            sp_base = tot - tot[first][g_of]
            sw = w2[sp] - CAP * TPE + sp_base[blk[sp]]
            assert sw.max(initial=0) < TPE, ("spill overflow",
                                             int(sw.max(initial=0)))
            sl[sp] = spill_tile_id[g_of[blk[sp]]] * TPE + sw
            do = np.where(is_base, dl % BLKx, dl % GRP).astype(np.float32)
            percore.append((eidx, sl, do))
        return dict(chunks=chunks, tiles=tiles, T=T, Tpad=Tpad,
                    NPAGE=npage, page_tiles=page_tiles, percore=percore,
                    bpg=bpg)

    L1 = capspill(BLK1, CAP1)
    L2 = capspill(BLK2, CAP2)

    nd = norm_s * inv_deg[dst_s]
    c1 = (att1[et_s] * nd[:, None]).astype(np.float32)
    c2 = (att2[et_s] * nd[:, None]).astype(np.float32)

    # layer-2 gather: chunk-major hfull pair-row index for each source node
    k_s = src_s // NPC
    n_s = src_s % NPC
    g_s = n_s // GRP
    c_s = chunk_of_group[g_s]
    lpr = (g_s - chunk_gstart[c_s]) * (GRP // 2) + (n_s % GRP) // 2
    row_s = hfull_base[c_s] + k_s * np.asarray(pr_c)[c_s] + lpr
    par_s = (n_s % 2).astype(np.float32)

    def pack_idx(lin, npage):
        """[npage*GP*TPE] indices -> [npage, 128, GP*8] int16 layout."""
        out = np.empty((npage, 16, GP * 8), np.int16)
        lp = lin.reshape(npage, GP * TPE)
        out[:, :, :] = lp.reshape(npage, GP * 8, 16).transpose(0, 2, 1)
        return np.ascontiguousarray(np.tile(out, (1, 8, 1)))

    def scat(vals, sl, Tpad, fill=0.0, width=None):
        if width is None:
            out = np.full(Tpad * TPE, fill, dtype=np.float32)
        else:
            out = np.full((Tpad * TPE, width), fill, dtype=np.float32)
        out[sl] = vals
        return out

    def tilemaj(a, npage, w):
        """[Tpad*TPE(,w)] slot-major -> [npage, TPE, GP*w] tile-major."""
        if a.ndim == 1:
            a = a[:, None]
        return np.ascontiguousarray(
            a.reshape(npage, GP, TPE, a.shape[1]).transpose(0, 2, 1, 3)
            .reshape(npage, TPE, GP * a.shape[1]).astype(BF))

    ent = np.asarray(entity, dtype=np.float32)

    metac1s, metac2s, idx_pages, xpgs = [], [], [], []
    for m in range(M):
        eidx, sl1, do1 = L1["percore"][m]
        NP1, TP1 = L1["NPAGE"], L1["Tpad"]
        xsrcm = np.zeros((TP1 * TPE, D), dtype=BF)
        xsrcm[sl1] = ent[src_s[eidx]].astype(BF)
        xpgs.append(tilemaj(xsrcm.astype(np.float32), NP1, D))
        metac1 = np.concatenate([
            tilemaj(scat(do1, sl1, TP1, 320.0), NP1, 1),
            tilemaj(scat(c1[eidx], sl1, TP1, width=4), NP1, 4)], axis=2)
        metac1s.append(np.ascontiguousarray(metac1))

        eidx2, sl2, do2 = L2["percore"][m]
        NP2, TP2 = L2["NPAGE"], L2["Tpad"]
        rows = np.full(TP2 * TPE, zero_row, dtype=np.int64)
        rows[sl2] = row_s[eidx2]
        idx_pages.append(pack_idx(rows.astype(np.int16), NP2))
        par = scat(par_s[eidx2], sl2, TP2)
        ca = scat(c2[eidx2], sl2, TP2, width=4)
        metac2 = np.concatenate([
            tilemaj(scat(do2, sl2, TP2, 320.0), NP2, 1),
            tilemaj(ca * (1.0 - par)[:, None], NP2, 4),
            tilemaj(ca * par[:, None], NP2, 4)], axis=2)
        metac2s.append(np.ascontiguousarray(metac2))

    return dict(NPC=NPC, SEG=SEG, NTAB=NTAB, NGRP=NGRP, L1=L1, L2=L2,
                cg=cg, chunk_of_group=chunk_of_group,
                chunk_gstart=chunk_gstart, pr_c=pr_c, hfull_base=hfull_base,
                idxp=idx_pages, metac1=metac1s, metac2=metac2s, xpg=xpgs)


def _chunks_by_page(L):
    out = [[] for _ in range(L["NPAGE"])]
    for kind, chks, toff in L["chunks"]:
        out[toff // GP].append((kind, chks, toff))
    return out


def _make_prepper(nc, prm, gath2, table_ap, idxt_all, gsems, psems):
    """Prepare-only gather desc-gen for the first PREP_PAGES layer-2 pages.

    Each page's NSUB sub-preps are spread round-robin across the 4 SWDGE
    queues (keeps parked-descriptor count per ring low and lets the 4 Q7
    core pairs generate concurrently). trigger(q) fires page q's subs
    (count=1 on each queue, FIFO order); gwait(q) gates the tensor engine
    on the page's gather DMAs. Later pages use plain direct gathers."""
    page_tiles = prm["L2"]["page_tiles"]
    gbufs = {}
    nsub_q = [0, 0, 0, 0]       # subs emitted per queue
    ntrig_q = [0, 0, 0, 0]      # subs triggered per queue

    def emit_page_preps(q):
        if q not in gbufs:
            gbufs[q] = gath2.tile([TPE, GP, 2 * DW], BF16, tag="gbuf2",
                                  bufs=GB_BUFS, name="gbuf2")
        for si, (t0, t1) in enumerate(_page_subranges(page_tiles[q])):
            qq = si % 4
            nidx = (t1 - t0) * TPE
            nc.gpsimd.dma_gather(
                out_ap=gbufs[q][:, t0:t1, :], in_ap=table_ap,
                idxs_ap=idxt_all[:, q, t0 * 8:t1 * 8],
                num_idxs=nidx, num_idxs_reg=nidx,
                elem_size=2 * DW, single_packet=False, queue_num=qq,
                prepare_only=True, sem=gsems[qq]).then_inc(psems[qq], 1)
            nsub_q[qq] += 1

    def emit_direct(q):
        qq = q % 2
        if q not in gbufs:
            gbufs[q] = gath2.tile([TPE, GP, 2 * DW], BF16, tag="gbuf2",
                                  bufs=GB_BUFS, name="gbuf2")
        nt = page_tiles[q]
        nidx = nt * TPE
        nc.gpsimd.dma_gather(
            out_ap=gbufs[q][:, 0:nt, :], in_ap=table_ap,
            idxs_ap=idxt_all[:, q, 0:nt * 8],
            num_idxs=nidx, num_idxs_reg=nidx,
            elem_size=2 * DW, single_packet=False, queue_num=qq)

    def trigger(q):
        nsubs = len(_page_subranges(page_tiles[q]))
        for si in range(nsubs):
            qq = si % 4
            ntrig_q[qq] += 1
            nc.gpsimd.wait_ge(psems[qq], ntrig_q[qq])
            nc.gpsimd.trigger_dma(count=1, queue_num=qq)

    def gwait(q):
        for qq in range(4):
            nc.tensor.wait_ge(gsems[qq], 16 * (q + 1))

    return emit_page_preps, trigger, gbufs, emit_direct, gwait


def _layer1(tc, nc, pools, prm, D, xpg, metac1_all, iota1_t, iota128_t,
            ident_t, bas_t, rt_t, bias_t, xslb, h4buf, hsl_cs, group_done,
            prep_hook):
    """Dense layer 1: pre-gathered x rows; per-group spill + base tiles."""
    gath, wohp, hp, xtp, sbigp, psp = pools
    L = prm["L1"]
    NPAGE, tiles, bpg = L["NPAGE"], L["tiles"], L["bpg"]
    cbp = _chunks_by_page(L)

    pending = []

    def flush(now):
        while pending and (now is None or pending[0][0] + TAIL_DELAY <= now):
            _, g, sbig = pending.pop(0)
            _tail1(tc, nc, pools, prm, D, g, sbig, xslb, h4buf,
                   ident_t, bas_t, rt_t, bias_t, hsl_cs)
            group_done(g)
            prep_hook()

    psums = {}
    cglob = 0
    for q in range(NPAGE):
        gbuf = gath.tile([TPE, GP * D], BF16, tag="gbuf1", bufs=3)
        nc.sync.dma_start(out=gbuf[:], in_=xpg[q])
        metat = metac1_all[:, q, :]
        for kind, chks, toff in cbp[q]:
            flush(cglob)
            cglob += 1
            loff = toff - q * GP
            if kind == "s":
                g = tiles[toff][1]
                ohs = wohp.tile([TPE, GRP], BF16, tag="ohs1", bufs=3)
                nc.vector.tensor_tensor(
                    out=ohs[:], in0=iota128_t,
                    in1=_expand(metat, [[0, GRP]], col_offset=loff),
                    op=ALU.is_equal)
                wohs = wohp.tile([TPE, 4 * GRP], BF16, tag="wohs1", bufs=3)
                for b in range(4):
                    nc.vector.tensor_tensor(
                        out=_expand(wohs[:], [[4 * BLK1, bpg], [1, BLK1]],
                                    col_offset=b * BLK1),
                        in0=_expand(ohs[:], [[BLK1, bpg], [1, BLK1]]),
                        in1=_expand(metat, [[0, bpg], [0, BLK1]],
                                    col_offset=GP + loff * 4 + b),
                        op=ALU.mult)
                psum_blk = psp.tile([D, 4 * GRP], F32, tag="blk", bufs=3,
                                    name="psum_blk")
                psums[g] = psum_blk
                nc.tensor.matmul(out=psum_blk[:],
                                 lhsT=gbuf[:, loff * D:(loff + 1) * D],
                                 rhs=wohs[:], start=True, stop=False)
            else:
                oh8 = wohp.tile([TPE, CHK * BLK1], BF16, tag="oh1", bufs=4)
                nc.vector.tensor_tensor(
                    out=_expand(oh8[:], [[BLK1, CHK], [1, BLK1]]),
                    in0=iota1_t,
                    in1=_expand(metat, [[1, CHK], [0, BLK1]],
                                col_offset=loff),
                    op=ALU.is_equal)
                woh = wohp.tile([TPE, CHK * 4 * BLK1], BF16, tag="woh1",
                                bufs=4)
                for b in range(4):
                    nc.vector.tensor_tensor(
                        out=_expand(woh[:], [[4 * BLK1, CHK], [1, BLK1]],
                                    col_offset=b * BLK1),
                        in0=_expand(oh8[:], [[BLK1, CHK], [1, BLK1]]),
                        in1=_expand(metat, [[4, CHK], [0, BLK1]],
                                    col_offset=GP + loff * 4 + b),
                        op=ALU.mult)
                for u in range(CHK):
                    _, g, bi, w = tiles[toff + u]
                    last = (bi == bpg - 1 and w == CAP1 - 1)
                    nc.tensor.matmul(
                        out=psums[g][:, bi * 4 * BLK1:(bi + 1) * 4 * BLK1],
                        lhsT=gbuf[:, (loff + u) * D:(loff + u + 1) * D],
                        rhs=woh[:, u * 4 * BLK1:(u + 1) * 4 * BLK1],
                        start=False, stop=last)
                    if last:
                        psum_blk = psums.pop(g)
                        sbig = sbigp.tile([D, 4 * GRP], BF16, tag="sbig",
                                          bufs=5)
                        for b in range(4):
                            nc.scalar.copy(
                                out=_expand(sbig[:], [[BLK1, bpg], [1, BLK1]],
                                            col_offset=b * GRP),
                                in_=_expand(psum_blk[:],
                                            [[4 * BLK1, bpg], [1, BLK1]],
                                            col_offset=b * BLK1))
                        pending.append((cglob, g, sbig))
    flush(None)


def _tail1(tc, nc, pools, prm, D, g, sbig, xslb, h4buf, ident_t,
           bas_t, rt_t, bias_t, hsl_cs):
    """Layer-1 group tail: combine bases, mean, root, bias, relu -> hsl."""
    gath, wohp, hp, xtp, sbigp, psp = pools
    pagg = psp.tile([GRP, D], F32, tag="agg", bufs=2)
    for b in range(4):
        nc.tensor.matmul(
            out=pagg[:],
            lhsT=sbig[:, b * GRP:(b + 1) * GRP],
            rhs=bas_t[:, b * D:(b + 1) * D],
            start=(b == 0),
            stop=False,
        )
    nc.tensor.matmul(out=pagg[:], lhsT=xslb[:, g, :], rhs=rt_t,
                     start=False, stop=True)

    h3 = hp.tile([GRP, D], F32, tag="h3")
    nc.vector.tensor_tensor(out=h3[:], in0=pagg[:], in1=bias_t, op=ALU.add)

    nc.scalar.activation(out=h4buf[:, g, 0:D], in_=h3[:], func=AF.Relu)
    c = int(prm["chunk_of_group"][g])
    gl = g - int(prm["chunk_gstart"][c])
    base = hsl_cs[c][:]
    dst = bass.AP(base.tensor, base.offset + gl * (GRP // 2) * (2 * DW),
                  [[2 * DW, GRP // 2], [DW, 2], [1, DW]])
    nc.sync.dma_start(out=dst, in_=h4buf[:, g, :])


def _layer2(tc, nc, pools, prm, D, metat_all,
            iota2_t, iota128_t, ident_t, bas_t, rt_t, bias_t, h4buf,
            out_ap, trigger, gbufs, gwait, emit_direct):
    """Gather-based layer 2: per-group spill + 8 base tiles of 16."""
    gath, wohp, hp, xtp, sbigp, psp = pools
    L = prm["L2"]
    NPAGE, tiles, bpg = L["NPAGE"], L["tiles"], L["bpg"]
    cbp = _chunks_by_page(L)

    pending = []

    def flush(now):
        while pending and (now is None or pending[0][0] + TAIL_DELAY2 <= now):
            _, g, sbig = pending.pop(0)
            _tail2(tc, nc, pools, prm, D, g, sbig, h4buf, ident_t,
                   bas_t, rt_t, bias_t, out_ap)

    for q in range(min(GB_BUFS, NPAGE)):
        emit_direct(q)

    psums = {}
    cglob = 0
    for q in range(NPAGE):
        if q > 0 and q - 1 + GB_BUFS < NPAGE:
            emit_direct(q - 1 + GB_BUFS)
        metat = metat_all[:, q, :]
        gbuf = gbufs[q]
        for kind, chks, toff in cbp[q]:
            flush(cglob)
            cglob += 1
            loff = toff - q * GP
            if kind == "s":
                g = tiles[toff][1]
                ohs = wohp.tile([TPE, GRP], BF16, tag="ohs2", bufs=3)
                nc.vector.tensor_tensor(
                    out=ohs[:], in0=iota128_t,
                    in1=_expand(metat, [[0, GRP]], col_offset=loff),
                    op=ALU.is_equal)
                wohsE = wohp.tile([TPE, 4 * GRP], BF16, tag="wohsE", bufs=3)
                wohsO = wohp.tile([TPE, 4 * GRP], BF16, tag="wohsO", bufs=3)
                for woh, cbase in ((wohsE, GP), (wohsO, 5 * GP)):
                    for b in range(4):
                        nc.vector.tensor_tensor(
                            out=_expand(woh[:], [[4 * BLK2, bpg], [1, BLK2]],
                                        col_offset=b * BLK2),
                            in0=_expand(ohs[:], [[BLK2, bpg], [1, BLK2]]),
                            in1=_expand(metat, [[0, bpg], [0, BLK2]],
                                        col_offset=cbase + loff * 4 + b),
                            op=ALU.mult)
                psum_blk = psp.tile([D, 4 * GRP], F32, tag="blk", bufs=3,
                                    name="psum_blk")
                psums[g] = psum_blk
                nc.tensor.matmul(out=psum_blk[:], lhsT=gbuf[:, loff, 0:D],
                                 rhs=wohsE[:], start=True, stop=False)
                nc.tensor.matmul(out=psum_blk[:],
                                 lhsT=gbuf[:, loff, DW:DW + D],
                                 rhs=wohsO[:], start=False, stop=False)
            else:
                oh8 = wohp.tile([TPE, CHK * BLK2], BF16, tag="oh2", bufs=5)
                nc.vector.tensor_tensor(
                    out=_expand(oh8[:], [[BLK2, CHK], [1, BLK2]]),
                    in0=iota2_t,
                    in1=_expand(metat, [[1, CHK], [0, BLK2]],
                                col_offset=loff),
                    op=ALU.is_equal)
                wohE = wohp.tile([TPE, CHK * 4 * BLK2], BF16, tag="wohE",
                                 bufs=4)
                wohO = wohp.tile([TPE, CHK * 4 * BLK2], BF16, tag="wohO",
                                 bufs=4)
                for woh, cbase in ((wohE, GP), (wohO, 5 * GP)):
                    for b in range(4):
                        nc.vector.tensor_tensor(
                            out=_expand(woh[:], [[4 * BLK2, CHK], [1, BLK2]],
                                        col_offset=b * BLK2),
                            in0=_expand(oh8[:], [[BLK2, CHK], [1, BLK2]]),
                            in1=_expand(metat, [[4, CHK], [0, BLK2]],
                                        col_offset=cbase + loff * 4 + b),
                            op=ALU.mult)
                for u in range(CHK):
                    _, g, bi, w = tiles[toff + u]
                    stop = (bi == bpg - 1)
                    out_sl = psums[g][:, bi * 4 * BLK2:(bi + 1) * 4 * BLK2]
                    nc.tensor.matmul(out=out_sl,
                                     lhsT=gbuf[:, loff + u, 0:D],
                                     rhs=wohE[:, u * 4 * BLK2:(u + 1) * 4 * BLK2],
                                     start=False, stop=False)
                    nc.tensor.matmul(out=out_sl,
                                     lhsT=gbuf[:, loff + u, DW:DW + D],
                                     rhs=wohO[:, u * 4 * BLK2:(u + 1) * 4 * BLK2],
                                     start=False, stop=stop)
                    if stop:
                        psum_blk = psums.pop(g)
                        sbig = sbigp.tile([D, 4 * GRP], BF16, tag="sbig",
                                          bufs=5)
                        for b in range(4):
                            nc.scalar.copy(
                                out=_expand(sbig[:], [[BLK2, bpg], [1, BLK2]],
                                            col_offset=b * GRP),
                                in_=_expand(psum_blk[:],
                                            [[4 * BLK2, bpg], [1, BLK2]],
                                            col_offset=b * BLK2))
                        pending.append((cglob, g, sbig))
    flush(None)


def _tail2(tc, nc, pools, prm, D, g, sbig, h4buf, ident_t,
           bas_t, rt_t, bias_t, out_ap):
    gath, wohp, hp, xtp, sbigp, psp = pools
    ptr = psp.tile([D, GRP], BF16, tag="tr", bufs=2)
    nc.tensor.transpose(out=ptr[:], in_=h4buf[:, g, 0:D], identity=ident_t)
    xt = xtp.tile([D, GRP], BF16, tag="xt")
    nc.scalar.copy(out=xt[:], in_=ptr[:])

    pagg = psp.tile([GRP, D], F32, tag="agg", bufs=2)
    for b in range(4):
        nc.tensor.matmul(
            out=pagg[:],
            lhsT=sbig[:, b * GRP:(b + 1) * GRP],
            rhs=bas_t[:, b * D:(b + 1) * D],
            start=(b == 0),
            stop=False,
        )
    nc.tensor.matmul(out=pagg[:], lhsT=xt[:], rhs=rt_t, start=False, stop=True)

    h3 = hp.tile([GRP, D], F32, tag="h3")
    nc.vector.tensor_tensor(out=h3[:], in0=pagg[:], in1=bias_t, op=ALU.add)
    nc.scalar.dma_start(out=out_ap[g * GRP:(g + 1) * GRP, :], in_=h3[:])


def _build(prm, D):
    NPC, NTAB = prm["NPC"], prm["NTAB"]
    NGRP = prm["NGRP"]
    NPAGE1, NPAGE2 = prm["L1"]["NPAGE"], prm["L2"]["NPAGE"]
    pr_c, cg = prm["pr_c"], prm["cg"]
    nc = bacc.Bacc(num_swdge_queues=4)

    xslp = nc.dram_tensor("xslp", [D, NGRP * GRP], BF16, kind="ExternalInput")
    xpg = nc.dram_tensor("xpg", [NPAGE1, TPE, GP * D], BF16,
                         kind="ExternalInput")
    idxp = nc.dram_tensor("idxp", [NPAGE2, TPE, 8 * GP], I16,
                          kind="ExternalInput")
    metac1 = nc.dram_tensor("metac1", [NPAGE1, TPE, 5 * GP], BF16,
                            kind="ExternalInput")
    metac2 = nc.dram_tensor("metac2", [NPAGE2, TPE, 9 * GP], BF16,
                            kind="ExternalInput")
    iota1 = nc.dram_tensor("iota1", [TPE, CHK * BLK1], BF16,
                           kind="ExternalInput")
    iota2 = nc.dram_tensor("iota2", [TPE, CHK * BLK2], BF16,
                           kind="ExternalInput")
    iota128 = nc.dram_tensor("iota128", [TPE, GRP], BF16,
                             kind="ExternalInput")
    ident = nc.dram_tensor("ident", [TPE, TPE], BF16, kind="ExternalInput")
    bas1 = nc.dram_tensor("bas1", [D, 4 * D], BF16, kind="ExternalInput")
    bas2 = nc.dram_tensor("bas2", [D, 4 * D], BF16, kind="ExternalInput")
    rt1 = nc.dram_tensor("rt1", [D, D], BF16, kind="ExternalInput")
    rt2 = nc.dram_tensor("rt2", [D, D], BF16, kind="ExternalInput")
    bias1 = nc.dram_tensor("bias1", [GRP, D], F32, kind="ExternalInput")
    bias2 = nc.dram_tensor("bias2", [GRP, D], F32, kind="ExternalInput")
    outp = nc.dram_tensor("outp", [NPC, D], F32, kind="ExternalOutput")

    with tile.TileContext(nc) as tc:
        with (
            tc.tile_pool(name="const", bufs=1) as cst,
            tc.tile_pool(name="gath", bufs=3) as gath,
            tc.tile_pool(name="woh", bufs=3) as wohp,
            tc.tile_pool(name="hp", bufs=3) as hp,
            tc.tile_pool(name="xtp", bufs=3) as xtp,
            tc.tile_pool(name="sbig", bufs=2) as sbigp,
            tc.tile_pool(name="ps", bufs=1, space="PSUM") as psp,
            tc.tile_pool(name="dram", bufs=1, space="DRAM") as dramp,
        ):
            pools = (gath, wohp, hp, xtp, sbigp, psp)

            iota1_t = cst.tile([TPE, CHK * BLK1], BF16)
            nc.sync.dma_start(out=iota1_t[:], in_=iota1[:])
            iota2_t = cst.tile([TPE, CHK * BLK2], BF16)
            nc.sync.dma_start(out=iota2_t[:], in_=iota2[:])
            iota128_t = cst.tile([TPE, GRP], BF16)
            nc.sync.dma_start(out=iota128_t[:], in_=iota128[:])
            xslb = cst.tile([D, NGRP, GRP], BF16)
            nc.sync.dma_start(out=xslb[:], in_=xslp[:])
            h4buf = cst.tile([GRP, NGRP, DW], BF16)
            nc.vector.memset(h4buf[:], 0.0)
            ident_t = cst.tile([TPE, TPE], BF16)
            nc.sync.dma_start(out=ident_t[:], in_=ident[:])
            bas1_t = cst.tile([D, 4 * D], BF16)
            nc.sync.dma_start(out=bas1_t[:], in_=bas1[:])
            bas2_t = cst.tile([D, 4 * D], BF16)
            nc.sync.dma_start(out=bas2_t[:], in_=bas2[:])
            rt1_t = cst.tile([D, D], BF16)
            nc.sync.dma_start(out=rt1_t[:], in_=rt1[:])
            rt2_t = cst.tile([D, D], BF16)
            nc.sync.dma_start(out=rt2_t[:], in_=rt2[:])
            bias1_t = cst.tile([GRP, D], F32)
            nc.sync.dma_start(out=bias1_t[:], in_=bias1[:])
            bias2_t = cst.tile([GRP, D], F32)
            nc.sync.dma_start(out=bias2_t[:], in_=bias2[:])
            zed_t = cst.tile([GRP, 2 * DW], BF16)
            nc.gpsimd.memset(zed_t[:], 0.0)

            # preloaded layer-2 gather indices + both layers' meta pages
            idxt_all = cst.tile([TPE, NPAGE2, 8 * GP], I16)
            ib = idxp[:]
            nc.sync.dma_start(
                out=idxt_all[:],
                in_=bass.AP(ib.tensor, ib.offset,
                            [[8 * GP, TPE], [TPE * 8 * GP, NPAGE2],
                             [1, 8 * GP]]))
            metat2_all = cst.tile([TPE, NPAGE2, 9 * GP], BF16)
            mb2 = metac2[:]
            nc.sync.dma_start(
                out=metat2_all[:],
                in_=bass.AP(mb2.tensor, mb2.offset,
                            [[9 * GP, TPE], [TPE * 9 * GP, NPAGE2],
                             [1, 9 * GP]]))
            metac1_all = cst.tile([TPE, NPAGE1, 5 * GP], BF16)
            mb1 = metac1[:]
            nc.sync.dma_start(
                out=metac1_all[:],
                in_=bass.AP(mb1.tensor, mb1.offset,
                            [[5 * GP, TPE], [TPE * 5 * GP, NPAGE1],
                             [1, 5 * GP]]))

            hsl_cs = [dramp.tile([pr_c[c], 2 * DW], BF16, name=f"hsl_c{c}")
                      for c in range(NCHUNK)]
            hfull = dramp.tile([NTAB // 2, 2 * DW], BF16)

            # zero rows at the tail of chunk 0 (dummy gather targets)
            nc.sync.dma_start(
                out=hsl_cs[0][pr_c[0] - GRP // 2:pr_c[0], :],
                in_=zed_t[0:GRP // 2, :])

            gsems = [nc.alloc_semaphore(f"gsem{q}") for q in range(4)]
            psems = [nc.alloc_semaphore(f"psem{q}") for q in range(4)]
            for s in gsems + psems:
                nc.gpsimd.sem_clear(s)
            hchk = cst.tile([1, 2 * DW], BF16)
            hchk2 = cst.tile([1, 2 * DW], BF16)

            fired = [False] * NCHUNK
            last_group_of_chunk = np.cumsum(cg) - 1

            def group_done(g):
                for c in range(NCHUNK):
                    if g == last_group_of_chunk[c] and not fired[c]:
                        fired[c] = True
                        base = int(prm["hfull_base"][c] // 1)
                        rows = pr_c[c] * M
                        nc.gpsimd.collective_compute(
                            "AllGather",
                            ALU.bypass,
                            replica_groups=[list(range(M))],
                            ins=[hsl_cs[c][:]],
                            outs=[hfull[base:base + rows, :]],
                        )

            with tc.tile_pool(name="gath2", bufs=GB_BUFS) as gath2:
                (emit_page_preps, trigger, gbufs, emit_direct,
                 gwait) = _make_prepper(
                    nc, prm, gath2, hfull[:, :], idxt_all, gsems, psems)

                prep_q = []

                def prep_hook():
                    pass

                _layer1(tc, nc, pools, prm, D, xpg, metac1_all,
                        iota1_t[:], iota128_t[:], ident_t[:], bas1_t,
                        rt1_t[:], bias1_t[:], xslb, h4buf, hsl_cs,
                        group_done, prep_hook)

                _layer2(tc, nc, pools, prm, D, metat2_all,
                        iota2_t[:], iota128_t[:], ident_t[:], bas2_t,
                        rt2_t[:], bias2_t[:], h4buf, outp,
                        trigger, gbufs, gwait, emit_direct)
    nc.compile()
    return nc


def kernel(entity, edge_index, edge_attr, edge_type, edge_norm,
           basis1, att1, root1, bias1, basis2, att2, root2, bias2):
    N, D = entity.shape
    prm = _prep(N, D, np.asarray(entity), np.asarray(edge_index),
                np.asarray(edge_type), np.asarray(edge_norm),
                np.asarray(att1), np.asarray(att2))
    NPC = prm["NPC"]

    entity = np.asarray(entity, dtype=np.float32)

    iota1_arr = np.tile(np.arange(BLK1, dtype=np.float32), (TPE, CHK)).astype(BF)
    iota2_arr = np.tile(np.arange(BLK2, dtype=np.float32), (TPE, CHK)).astype(BF)
    iota128_arr = np.tile(np.arange(GRP, dtype=np.float32), (TPE, 1)).astype(BF)
    ident_arr = np.eye(TPE, dtype=np.float32).astype(BF)
    b1 = np.ascontiguousarray(
        np.asarray(basis1, np.float32).transpose(1, 0, 2).reshape(D, 4 * D)).astype(BF)
    b2 = np.ascontiguousarray(
        np.asarray(basis2, np.float32).transpose(1, 0, 2).reshape(D, 4 * D)).astype(BF)

    nc = _build(prm, D)

    in_maps = []
    for m in range(M):
        lo, hi = m * NPC, min((m + 1) * NPC, N)
        xs = np.zeros((NPC, D), dtype=np.float32)
        if hi > lo:
            xs[0:hi - lo] = entity[lo:hi]
        xslp_arr = np.ascontiguousarray(
            xs.reshape(prm["NGRP"], GRP, D).transpose(2, 0, 1)
            .reshape(D, prm["NGRP"] * GRP)).astype(BF)
        in_maps.append({
            "xslp": xslp_arr,
            "xpg": prm["xpg"][m],
            "idxp": prm["idxp"][m],
            "metac1": prm["metac1"][m],
            "metac2": prm["metac2"][m],
            "iota1": iota1_arr,
            "iota2": iota2_arr,
            "iota128": iota128_arr,
            "ident": ident_arr,
            "bas1": b1,
            "bas2": b2,
            "rt1": np.asarray(root1, np.float32).astype(BF),
            "rt2": np.asarray(root2, np.float32).astype(BF),
            "bias1": np.tile(np.asarray(bias1, np.float32), (GRP, 1)),
            "bias2": np.tile(np.asarray(bias2, np.float32), (GRP, 1)),
        })
    kwargs = {}
    if TRACE:
        kwargs = dict(trace=True, tmpdir=TRACE_DIR)
    res = run_bass_kernel_spmd(nc, in_maps, core_ids=list(range(M)), **kwargs)
    global LAST
    LAST = res
    out = np.concatenate([res.results[m]["outp"] for m in range(M)], axis=0)
    return np.ascontiguousarray(out[:N])


LAST = None
TRACE = False
TRACE_DIR = None
